# revision 5
# baseline (speedup 1.0000x reference)
"""BertSelfAttention (softsign-modified) Trainium2 Bass kernel.

Sharding: 8 cores = 2 batches x 4 head-groups (3 heads each).
Host gathers unmasked queries (mask applies along the QUERY dim only:
masked rows get uniform softmax => output = mean(V), filled host-side).

Device per core (all fp32):
  - proj: qT/kT/vT = W_hT.T @ hiddenT (hiddenT streamed in slabs)
  - k_mod = k/8 + softsign(softsign(k)/8) + v  (DVE fused ops;
    ss(ss1/8) = ss1/(8+|ss1|))
  - scores^T[k,q] = km^T.T @ qT  (two heads packed in partition halves,
    row-tiled concurrent matmuls)
  - probs = exp(scores/8) on ACT (no max subtraction needed; |s/8|<~8)
  - ctx^T[d,q] + sumexp via lhsT=[V|ones] accumulation over k tiles
  - PE-transpose ctx^T -> ctx, per-partition reciprocal normalize
  - meanV per head appended as the last output row
"""

import functools
import os
import sys

import numpy as np

for _p in ("/opt/trn_rl_repo", "/root/.axon_site/_ro/trn_rl_repo"):
    if os.path.isdir(_p) and _p not in sys.path:
        sys.path.append(_p)

import concourse.bacc as bacc
import concourse.mybir as mybir
import concourse.tile as tile
from concourse import bass_utils

F32 = mybir.dt.float32
ALU = mybir.AluOpType
ACTF = mybir.ActivationFunctionType

B, S, HD, H, D = 2, 4096, 768, 12, 64
NCORES = 8
HPC = 3  # heads per core
QB = 512  # q block (one PSUM bank of fp32)
KT = 128  # k tile (partition dim of scores^T)
NB = 512  # projection N block
KCH = HD // 128  # 6 contraction chunks
NKT = S // KT  # 32 k tiles
SCALE = 0.125  # 1/sqrt(D)


def _emit(nc, tc, P_q, t):
    """Emit the tile program. t = dict of dram tensor APs."""
    nq = P_q // QB

    with (
        tc.tile_pool(name="persist", bufs=1) as P,
        tc.tile_pool(name="work", bufs=2) as W,
        tc.tile_pool(name="scr", bufs=5) as SCR,
        tc.tile_pool(name="probs", bufs=3) as PRB,
        tc.tile_pool(name="psA", bufs=2, space="PSUM") as psA,
        tc.tile_pool(name="psB", bufs=4, space="PSUM") as psB,
    ):
        # ---- persistent SBUF ----
        q01 = P.tile([128, P_q], F32)
        q22 = P.tile([128, P_q], F32)
        k01 = P.tile([128, S], F32)  # becomes km01 in place
        k22 = P.tile([128, S], F32)  # becomes km22 in place
        v01 = P.tile([128, S], F32)
        v22 = P.tile([128, S], F32)
        vn0 = P.tile([128, 65 * NKT], F32)  # V natural + ones col, head 0
        vn1 = P.tile([128, 65 * NKT], F32)
        vn2 = P.tile([128, 65 * NKT], F32)
        outst = P.tile([128, (P_q // 128) * 192], F32)
        ident = P.tile([128, 128], F32)
        mvsb = P.tile([1, 192], F32)

        nc.sync.dma_start(ident[:], t["ident"][:])

        wsb = {}
        bsb = {}
        for nm in ("q01", "q22", "k01", "k22", "v01", "v22"):
            wsb[nm] = P.tile([128, KCH * 128], F32, name=f"w_{nm}_sb")
            nc.sync.dma_start(
                wsb[nm][:].rearrange("p (c m) -> p c m", c=KCH),
                t[f"w_{nm}"][:].rearrange("(c p) m -> p c m", p=128),
            )
            bsb[nm] = P.tile([128, 1], F32, name=f"b_{nm}_sb")
            nc.sync.dma_start(bsb[nm][:], t[f"b_{nm}"][:])

        def proj_block(src_ap, nb, chains):
            slab = W.tile([128, KCH * NB], F32, tag="slab", name="slab")
            nc.sync.dma_start(
                slab[:].rearrange("p (c s) -> p c s", c=KCH),
                src_ap[:, nb * NB : (nb + 1) * NB].rearrange("(c p) s -> p c s", p=128),
            )
            for nm, dst in chains:
                ps = psB.tile([128, NB], F32, tag="cx", name="pp")
                for c in range(KCH):
                    nc.tensor.matmul(
                        ps[:],
                        wsb[nm][:, c * 128 : (c + 1) * 128],
                        slab[:, c * NB : (c + 1) * NB],
                        start=(c == 0),
                        stop=(c == KCH - 1),
                    )
                nc.vector.tensor_scalar_add(
                    dst[:, nb * NB : (nb + 1) * NB], ps[:], bsb[nm][:]
                )

        # ---- projections: k/v over full S, q over gathered P_q ----
        for nb in range(S // NB):
            proj_block(
                t["hT_full"],
                nb,
                [("k01", k01), ("k22", k22), ("v01", v01), ("v22", v22)],
            )
        for nb in range(nq):
            proj_block(t["hT_sel"], nb, [("q01", q01), ("q22", q22)])

        # ---- V natural (+ones col) via PE transpose ----
        for vn in (vn0, vn1, vn2):
            nc.gpsimd.memset(vn[:], 1.0)
        for tt in range(NKT):
            pt = psB.tile([128, 128], F32, tag="cx", name="pt")
            nc.tensor.transpose(pt[:], v01[:, tt * 128 : (tt + 1) * 128], ident[:])
            nc.vector.tensor_copy(vn0[:, tt * 65 : tt * 65 + 64], pt[:, 0:64])
            nc.vector.tensor_copy(vn1[:, tt * 65 : tt * 65 + 64], pt[:, 64:128])
            pt2 = psB.tile([128, 128], F32, tag="cx", name="pt2")
            nc.tensor.transpose(pt2[:], v22[:, tt * 128 : (tt + 1) * 128], ident[:])
            nc.vector.tensor_copy(vn2[:, tt * 65 : tt * 65 + 64], pt2[:, 0:64])

        # ---- k_mod (in place into k01/k22) ----
        CH = 1024
        for kbuf, vbuf in ((k01, v01), (k22, v22)):
            for ch in range(S // CH):
                sl = slice(ch * CH, (ch + 1) * CH)
                u = SCR.tile([128, CH], F32, tag="scr", name="u")
                nc.vector.scalar_tensor_tensor(
                    u[:], kbuf[:, sl], SCALE, vbuf[:, sl], op0=ALU.mult, op1=ALU.add
                )
                a1 = SCR.tile([128, CH], F32, tag="scr", name="a1")
                nc.scalar.activation(a1[:], kbuf[:, sl], ACTF.Abs)
                t1 = SCR.tile([128, CH], F32, tag="scr", name="t1")
                nc.vector.tensor_scalar_add(t1[:], a1[:], 1.0)
                r1 = SCR.tile([128, CH], F32, tag="scr", name="r1")
                nc.vector.reciprocal_approx_fast(r1[:], t1[:])
                s1 = SCR.tile([128, CH], F32, tag="scr", name="s1")
                nc.vector.tensor_mul(s1[:], kbuf[:, sl], r1[:])
                a2 = SCR.tile([128, CH], F32, tag="scr", name="a2")
                nc.scalar.activation(a2[:], s1[:], ACTF.Abs)
                t2 = SCR.tile([128, CH], F32, tag="scr", name="t2")
                nc.vector.tensor_scalar_add(t2[:], a2[:], 8.0)
                r2 = SCR.tile([128, CH], F32, tag="scr", name="r2")
                nc.vector.reciprocal_approx_fast(r2[:], t2[:])
                p1 = SCR.tile([128, CH], F32, tag="scr", name="p1")
                nc.vector.tensor_mul(p1[:], s1[:], r2[:])
                nc.vector.tensor_add(kbuf[:, sl], u[:], p1[:])

        # ---- attention ----
        def epilogue(ctx, col0, qi):
            cs = W.tile([65, QB], F32, tag="cs", name="cs")
            nc.vector.tensor_copy(cs[:], ctx[:])
            for tt in range(QB // 128):
                tp = psB.tile([128, 65], F32, tag="cx", name="tp")
                nc.tensor.transpose(
                    tp[:], cs[:, tt * 128 : (tt + 1) * 128], ident[0:65, 0:65]
                )
                rc = W.tile([128, 1], F32, tag="rc", name="rc")
                nc.vector.reciprocal(rc[:], tp[:, 64:65])
                g = qi * (QB // 128) + tt
                nc.vector.tensor_scalar_mul(
                    outst[:, g * 192 + col0 : g * 192 + col0 + 64], tp[:, 0:64], rc[:]
                )

        def attn_block(kmbuf, qbuf, qa, qb_, vnA, vnB, colA, colB):
            """One (slot0, slot1) pass over all k tiles for q blocks qa/qb_."""
            ctx0 = psB.tile([65, QB], F32, tag="cx", name="ctx0")
            ctx1 = psB.tile([65, QB], F32, tag="cx", name="ctx1")
            for kt in range(NKT):
                sc = psA.tile([128, 2 * QB], F32, tag="sc", name="sc")
                nc.tensor.matmul(
                    sc[:, 0:QB],
                    kmbuf[0:64, kt * KT : (kt + 1) * KT],
                    qbuf[0:64, qa * QB : (qa + 1) * QB],
                    start=True,
                    stop=True,
                )
                nc.tensor.matmul(
                    sc[:, QB : 2 * QB],
                    kmbuf[64:128, kt * KT : (kt + 1) * KT],
                    qbuf[64:128, qb_ * QB : (qb_ + 1) * QB],
                    start=True,
                    stop=True,
                )
                pb = PRB.tile([128, 2 * QB], F32, tag="pb", name="pb")
                nc.scalar.activation(pb[:], sc[:], ACTF.Exp, scale=SCALE)
                nc.tensor.matmul(
                    ctx0[:],
                    vnA[:, kt * 65 : kt * 65 + 65],
                    pb[:, 0:QB],
                    start=(kt == 0),
                    stop=(kt == NKT - 1),
                )
                nc.tensor.matmul(
                    ctx1[:],
                    vnB[:, kt * 65 : kt * 65 + 65],
                    pb[:, QB : 2 * QB],
                    start=(kt == 0),
                    stop=(kt == NKT - 1),
                )
            epilogue(ctx0, colA, qa)
            epilogue(ctx1, colB, qb_)

        # unit01: heads 0,1 share each q block (slot = head)
        for qi in range(nq):
            attn_block(k01, q01, qi, qi, vn0, vn1, 0, 64)

        # unit22: head 2 self-paired across q blocks (slot = q block)
        for st in range(nq // 2):
            attn_block(k22, q22, 2 * st, 2 * st + 1, vn2, vn2, 128, 128)
        if nq % 2:
            qt = nq - 1
            ctx0 = psB.tile([65, QB], F32, tag="cx", name="ctxT")
            for k2 in range(NKT // 2):
                ka, kb = 2 * k2, 2 * k2 + 1
                sc = psA.tile([128, 2 * QB], F32, tag="sc", name="sc")
                nc.tensor.matmul(
                    sc[:, 0:QB],
                    k22[0:64, ka * KT : (ka + 1) * KT],
                    q22[0:64, qt * QB : (qt + 1) * QB],
                    start=True,
                    stop=True,
                )
                nc.tensor.matmul(
                    sc[:, QB : 2 * QB],
                    k22[64:128, kb * KT : (kb + 1) * KT],
                    q22[64:128, qt * QB : (qt + 1) * QB],
                    start=True,
                    stop=True,
                )
                pb = PRB.tile([128, 2 * QB], F32, tag="pb", name="pb")
                nc.scalar.activation(pb[:], sc[:], ACTF.Exp, scale=SCALE)
                nc.tensor.matmul(
                    ctx0[:],
                    vn2[:, ka * 65 : ka * 65 + 65],
                    pb[:, 0:QB],
                    start=(k2 == 0),
                    stop=False,
                )
                nc.tensor.matmul(
                    ctx0[:],
                    vn2[:, kb * 65 : kb * 65 + 65],
                    pb[:, QB : 2 * QB],
                    start=False,
                    stop=(k2 == NKT // 2 - 1),
                )
            epilogue(ctx0, 128, qt)

        # ---- meanV row ----
        for h, vn in ((0, vn0), (1, vn1), (2, vn2)):
            mv = psB.tile([1, 64], F32, tag="cx", name="mv")
            for kt in range(NKT):
                nc.tensor.matmul(
                    mv[:],
                    vn[:, kt * 65 + 64 : kt * 65 + 65],
                    vn[:, kt * 65 : kt * 65 + 64],
                    start=(kt == 0),
                    stop=(kt == NKT - 1),
                )
            nc.vector.tensor_scalar_mul(mvsb[:, h * 64 : (h + 1) * 64], mv[:], 1.0 / S)
        nc.sync.dma_start(t["out"][P_q : P_q + 1, :], mvsb[:])

        # ---- store ----
        for g in range(P_q // 128):
            nc.sync.dma_start(
                t["out"][g * 128 : (g + 1) * 128, :],
                outst[:, g * 192 : (g + 1) * 192],
            )


@functools.lru_cache(maxsize=4)
def _build(P_q):
    nc = bacc.Bacc(
        "TRN2",
        target_bir_lowering=False,
        debug=False,
        enable_asserts=False,
        num_devices=NCORES,
    )
    t = {}
    t["hT_full"] = nc.dram_tensor("hT_full", [HD, S], F32, kind="ExternalInput").ap()
    t["hT_sel"] = nc.dram_tensor("hT_sel", [HD, P_q], F32, kind="ExternalInput").ap()
    for nm in ("q01", "q22", "k01", "k22", "v01", "v22"):
        t[f"w_{nm}"] = nc.dram_tensor(
            f"w_{nm}", [HD, 128], F32, kind="ExternalInput"
        ).ap()
        t[f"b_{nm}"] = nc.dram_tensor(
            f"b_{nm}", [128, 1], F32, kind="ExternalInput"
        ).ap()
    t["ident"] = nc.dram_tensor("ident", [128, 128], F32, kind="ExternalInput").ap()
    t["out"] = nc.dram_tensor("out", [P_q + 1, 192], F32, kind="ExternalOutput").ap()

    with tile.TileContext(nc) as tc:
        _emit(nc, tc, P_q, t)
    nc.compile()
    return nc


def _prep_core_inputs(hidden, sel_pad, Wq, bq, Wk, bk, Wv, bv, heads):
    """Build the in_map for one core. hidden: [S, HD] for this batch."""
    h0, h1, h2 = heads
    m = {}
    m["hT_full"] = np.ascontiguousarray(hidden.T)
    m["hT_sel"] = np.ascontiguousarray(hidden[sel_pad].T)

    def wT(Wmat, h):
        return np.ascontiguousarray(Wmat[h * D : (h + 1) * D, :].T)

    def bs(bvec, h):
        return bvec[h * D : (h + 1) * D]

    for nm, Wmat, bvec in (("q", Wq, bq), ("k", Wk, bk), ("v", Wv, bv)):
        m[f"w_{nm}01"] = np.concatenate([wT(Wmat, h0), wT(Wmat, h1)], axis=1)
        m[f"w_{nm}22"] = np.concatenate([wT(Wmat, h2), wT(Wmat, h2)], axis=1)
        m[f"b_{nm}01"] = np.concatenate([bs(bvec, h0), bs(bvec, h1)]).reshape(128, 1)
        m[f"b_{nm}22"] = np.concatenate([bs(bvec, h2), bs(bvec, h2)]).reshape(128, 1)
    m["ident"] = np.eye(128, dtype=np.float32)
    m = {k: np.ascontiguousarray(v, dtype=np.float32) for k, v in m.items()}
    return m


def _plan(attention_mask):
    """Returns (P_q, sel list, sel_pad list)."""
    sels = [np.where(attention_mask[b] != 0)[0] for b in range(B)]
    nmax = max(1, max(len(s) for s in sels))
    P_q = ((nmax + QB - 1) // QB) * QB
    sel_pads = []
    for s in sels:
        pad = np.zeros(P_q, dtype=np.int64)
        pad[: len(s)] = s
        sel_pads.append(pad)
    return P_q, sels, sel_pads


def build_in_maps(hidden_states, attention_mask, Wq, bq, Wk, bk, Wv, bv):
    P_q, sels, sel_pads = _plan(np.asarray(attention_mask))
    hs = np.asarray(hidden_states, dtype=np.float32)
    in_maps = []
    for c in range(NCORES):
        b, g = c // 4, c % 4
        heads = (3 * g, 3 * g + 1, 3 * g + 2)
        in_maps.append(
            _prep_core_inputs(hs[b], sel_pads[b], Wq, bq, Wk, bk, Wv, bv, heads)
        )
    return P_q, sels, in_maps


def assemble(results, P_q, sels, attention_mask):
    out = np.empty((B, S, HD), dtype=np.float32)
    mask = np.asarray(attention_mask)
    for c in range(NCORES):
        b, g = c // 4, c % 4
        r = results[c]["out"]
        cols = slice(192 * g, 192 * (g + 1))
        sel = sels[b]
        if len(sel):
            out[b, sel, cols] = r[: len(sel)]
        inv = np.where(mask[b] == 0)[0]
        if len(inv):
            out[b, inv, cols] = r[P_q]
    return out


def _install_ntff_shim():
    """Provide antenv.axon_hooks (missing from this image) so
    run_bass_kernel_spmd(trace=True) can capture NTFF profiles, and stub
    out the network-dependent artifact upload."""
    import types

    try:
        import antenv
    except ImportError:
        return
    try:
        from antenv.axon_hooks import get_axon_ntff_profile_hook  # noqa: F401
    except ImportError:
        try:
            if "/root/.axon_site" not in sys.path:
                sys.path.insert(0, "/root/.axon_site")
            from trn_agent_boot.trn_boot import _ntff_profile_via_ctypes

            hook = _ntff_profile_via_ctypes("/opt/axon/libaxon_pjrt.so")
        except Exception:
            hook = None
        mod = types.ModuleType("antenv.axon_hooks")
        _h = {"h": hook}
        mod.get_axon_ntff_profile_hook = lambda: _h["h"]
        mod.set_axon_ntff_profile_hook = lambda h: _h.__setitem__("h", h)
        sys.modules["antenv.axon_hooks"] = mod
        antenv.axon_hooks = mod

    _orig_upload = bass_utils.upload_artifacts

    def _safe_upload(tmpdir):
        try:
            return _orig_upload(tmpdir)
        except Exception:
            return tmpdir

    bass_utils.upload_artifacts = _safe_upload


def kernel(hidden_states, attention_mask, Wq, bq, Wk, bk, Wv, bv, trace=False):
    if trace:
        _install_ntff_shim()
    P_q, sels, in_maps = build_in_maps(
        hidden_states, attention_mask, Wq, bq, Wk, bk, Wv, bv
    )
    nc = _build(P_q)
    res = bass_utils.run_bass_kernel_spmd(
        nc, in_maps, core_ids=list(range(NCORES)), trace=trace
    )
    out = assemble(res.results, P_q, sels, attention_mask)
    if trace:
        kernel.last_exec_time_ns = res.exec_time_ns
        kernel.last_results = res
    return out


# revision 7
# speedup vs baseline: 1.8682x; 1.8682x over previous
"""BertSelfAttention (softsign-modified) Trainium2 Bass kernel.

Sharding: 8 cores = 2 batches x 4 head-groups (3 heads each).
Host gathers unmasked queries (mask applies along the QUERY dim only:
masked rows get uniform softmax => output = mean(V), filled host-side).

Device per core (all fp32):
  - proj: qT/kT/vT = W_hT.T @ hiddenT (hiddenT streamed in slabs)
  - k_mod = k/8 + softsign(softsign(k)/8) + v  (DVE fused ops;
    ss(ss1/8) = ss1/(8+|ss1|))
  - scores^T[k,q] = km^T.T @ qT  (two heads packed in partition halves,
    row-tiled concurrent matmuls)
  - probs = exp(scores/8) on ACT (no max subtraction needed; |s/8|<~8)
  - ctx^T[d,q] + sumexp via lhsT=[V|ones] accumulation over k tiles
  - PE-transpose ctx^T -> ctx, per-partition reciprocal normalize
  - meanV per head appended as the last output row
"""

import functools
import os
import sys

import numpy as np

for _p in ("/opt/trn_rl_repo", "/root/.axon_site/_ro/trn_rl_repo"):
    if os.path.isdir(_p) and _p not in sys.path:
        sys.path.append(_p)

import concourse.bacc as bacc
import concourse.mybir as mybir
import concourse.tile as tile
from concourse import bass_utils

F32 = mybir.dt.float32
BF16 = mybir.dt.float16  # 16-bit matmul dtype (fp16: 10-bit mantissa)
ALU = mybir.AluOpType
ACTF = mybir.ActivationFunctionType

B, S, HD, H, D = 2, 4096, 768, 12, 64
NCORES = 8
HPC = 3  # heads per core
QB = 512  # q block (one PSUM bank of fp32)
KT = 128  # k tile (partition dim of scores^T)
NB = 512  # projection N block
KCH = HD // 128  # 6 contraction chunks
NKT = S // KT  # 32 k tiles
SCALE = 0.125  # 1/sqrt(D)


def _emit(nc, tc, P_q, t):
    """Emit the tile program. t = dict of dram tensor APs."""
    nq = P_q // QB

    with (
        tc.tile_pool(name="persist", bufs=1) as P,
        tc.tile_pool(name="work", bufs=2) as W,
        tc.tile_pool(name="scr", bufs=5) as SCR,
        tc.tile_pool(name="probs", bufs=3) as PRB,
        tc.tile_pool(name="psA", bufs=2, space="PSUM") as psA,
        tc.tile_pool(name="psB", bufs=4, space="PSUM") as psB,
    ):
        # ---- persistent SBUF ----
        q01 = P.tile([128, P_q], BF16)
        q22 = P.tile([128, P_q], BF16)
        k01 = P.tile([128, S], F32)
        k22 = P.tile([128, S], F32)
        km01 = P.tile([128, S], BF16)
        km22 = P.tile([128, S], BF16)
        v01 = P.tile([128, S], F32)
        v22 = P.tile([128, S], F32)
        vn0 = P.tile([128, 65 * NKT], BF16)  # V natural + ones col, head 0
        vn1 = P.tile([128, 65 * NKT], BF16)
        vn2 = P.tile([128, 65 * NKT], BF16)
        outst = P.tile([128, (P_q // 128) * 192], F32)
        ident = P.tile([128, 128], F32)
        mvsb = P.tile([1, 192], F32)

        nc.sync.dma_start(ident[:], t["ident"][:])

        wsb = {}
        bsb = {}
        for nm in ("q01", "q22", "k01", "k22", "v01", "v22"):
            wsb[nm] = P.tile([128, KCH * 128], F32, name=f"w_{nm}_sb")
            nc.sync.dma_start(
                wsb[nm][:].rearrange("p (c m) -> p c m", c=KCH),
                t[f"w_{nm}"][:].rearrange("(c p) m -> p c m", p=128),
            )
            bsb[nm] = P.tile([128, 1], F32, name=f"b_{nm}_sb")
            nc.sync.dma_start(bsb[nm][:], t[f"b_{nm}"][:])

        def proj_block(src_ap, nb, chains):
            slab = W.tile([128, KCH * NB], F32, tag="slab", name="slab")
            nc.sync.dma_start(
                slab[:].rearrange("p (c s) -> p c s", c=KCH),
                src_ap[:, nb * NB : (nb + 1) * NB].rearrange("(c p) s -> p c s", p=128),
            )
            for nm, dst in chains:
                ps = psB.tile([128, NB], F32, tag="cx", name="pp")
                for c in range(KCH):
                    nc.tensor.matmul(
                        ps[:],
                        wsb[nm][:, c * 128 : (c + 1) * 128],
                        slab[:, c * NB : (c + 1) * NB],
                        start=(c == 0),
                        stop=(c == KCH - 1),
                    )
                nc.vector.tensor_scalar_add(
                    dst[:, nb * NB : (nb + 1) * NB], ps[:], bsb[nm][:]
                )

        # ---- projections: k/v over full S, q over gathered P_q ----
        for nb in range(S // NB):
            proj_block(
                t["hT_full"],
                nb,
                [("k01", k01), ("k22", k22), ("v01", v01), ("v22", v22)],
            )
        for nb in range(nq):
            proj_block(t["hT_sel"], nb, [("q01", q01), ("q22", q22)])

        # ---- V natural (+ones col) via PE transpose ----
        for vn in (vn0, vn1, vn2):
            nc.gpsimd.memset(vn[:], 1.0)
        for tt in range(NKT):
            pt = psB.tile([128, 128], F32, tag="cx", name="pt")
            nc.tensor.transpose(pt[:], v01[:, tt * 128 : (tt + 1) * 128], ident[:])
            nc.vector.tensor_copy(vn0[:, tt * 65 : tt * 65 + 64], pt[:, 0:64])
            nc.vector.tensor_copy(vn1[:, tt * 65 : tt * 65 + 64], pt[:, 64:128])
            pt2 = psB.tile([128, 128], F32, tag="cx", name="pt2")
            nc.tensor.transpose(pt2[:], v22[:, tt * 128 : (tt + 1) * 128], ident[:])
            nc.vector.tensor_copy(vn2[:, tt * 65 : tt * 65 + 64], pt2[:, 0:64])

        # ---- k_mod (in place into k01/k22) ----
        CH = 1024
        for kbuf, vbuf, kmbuf in ((k01, v01, km01), (k22, v22, km22)):
            for ch in range(S // CH):
                sl = slice(ch * CH, (ch + 1) * CH)
                u = SCR.tile([128, CH], F32, tag="scr", name="u")
                nc.vector.scalar_tensor_tensor(
                    u[:], kbuf[:, sl], SCALE, vbuf[:, sl], op0=ALU.mult, op1=ALU.add
                )
                a1 = SCR.tile([128, CH], F32, tag="scr", name="a1")
                nc.scalar.activation(a1[:], kbuf[:, sl], ACTF.Abs)
                t1 = SCR.tile([128, CH], F32, tag="scr", name="t1")
                nc.vector.tensor_scalar_add(t1[:], a1[:], 1.0)
                r1 = SCR.tile([128, CH], F32, tag="scr", name="r1")
                nc.vector.reciprocal_approx_fast(r1[:], t1[:])
                s1 = SCR.tile([128, CH], F32, tag="scr", name="s1")
                nc.vector.tensor_mul(s1[:], kbuf[:, sl], r1[:])
                a2 = SCR.tile([128, CH], F32, tag="scr", name="a2")
                nc.scalar.activation(a2[:], s1[:], ACTF.Abs)
                t2 = SCR.tile([128, CH], F32, tag="scr", name="t2")
                nc.vector.tensor_scalar_add(t2[:], a2[:], 8.0)
                r2 = SCR.tile([128, CH], F32, tag="scr", name="r2")
                nc.vector.reciprocal_approx_fast(r2[:], t2[:])
                p1 = SCR.tile([128, CH], F32, tag="scr", name="p1")
                nc.vector.tensor_mul(p1[:], s1[:], r2[:])
                nc.vector.tensor_add(kmbuf[:, sl], u[:], p1[:])

        # ---- attention ----
        def epilogue(ctx, col0, qi):
            cs = W.tile([65, QB], F32, tag="cs", name="cs")
            nc.vector.tensor_copy(cs[:], ctx[:])
            for tt in range(QB // 128):
                tp = psB.tile([128, 65], F32, tag="cx", name="tp")
                nc.tensor.transpose(
                    tp[:], cs[:, tt * 128 : (tt + 1) * 128], ident[0:65, 0:65]
                )
                rc = W.tile([128, 1], F32, tag="rc", name="rc")
                nc.vector.reciprocal(rc[:], tp[:, 64:65])
                g = qi * (QB // 128) + tt
                nc.vector.tensor_scalar_mul(
                    outst[:, g * 192 + col0 : g * 192 + col0 + 64], tp[:, 0:64], rc[:]
                )

        def attn_block(kmbuf, qbuf, qa, qb_, vnA, vnB, colA, colB):
            """One (slot0, slot1) pass over all k tiles for q blocks qa/qb_."""
            ctx0 = psB.tile([65, QB], F32, tag="cx", name="ctx0")
            ctx1 = psB.tile([65, QB], F32, tag="cx", name="ctx1")
            for kt in range(NKT):
                sc = psA.tile([128, 2 * QB], F32, tag="sc", name="sc")
                nc.tensor.matmul(
                    sc[:, 0:QB],
                    kmbuf[0:64, kt * KT : (kt + 1) * KT],
                    qbuf[0:64, qa * QB : (qa + 1) * QB],
                    start=True,
                    stop=True,
                )
                nc.tensor.matmul(
                    sc[:, QB : 2 * QB],
                    kmbuf[64:128, kt * KT : (kt + 1) * KT],
                    qbuf[64:128, qb_ * QB : (qb_ + 1) * QB],
                    start=True,
                    stop=True,
                )
                pb = PRB.tile([128, 2 * QB], BF16, tag="pb", name="pb")
                nc.scalar.activation(pb[:], sc[:], ACTF.Exp, scale=SCALE)
                nc.tensor.matmul(
                    ctx0[:],
                    vnA[:, kt * 65 : kt * 65 + 65],
                    pb[:, 0:QB],
                    start=(kt == 0),
                    stop=(kt == NKT - 1),
                )
                nc.tensor.matmul(
                    ctx1[:],
                    vnB[:, kt * 65 : kt * 65 + 65],
                    pb[:, QB : 2 * QB],
                    start=(kt == 0),
                    stop=(kt == NKT - 1),
                )
            epilogue(ctx0, colA, qa)
            epilogue(ctx1, colB, qb_)

        # unit01: heads 0,1 share each q block (slot = head)
        for qi in range(nq):
            attn_block(km01, q01, qi, qi, vn0, vn1, 0, 64)

        # unit22: head 2 self-paired across q blocks (slot = q block)
        for st in range(nq // 2):
            attn_block(km22, q22, 2 * st, 2 * st + 1, vn2, vn2, 128, 128)
        if nq % 2:
            qt = nq - 1
            ctx0 = psB.tile([65, QB], F32, tag="cx", name="ctxT")
            for k2 in range(NKT // 2):
                ka, kb = 2 * k2, 2 * k2 + 1
                sc = psA.tile([128, 2 * QB], F32, tag="sc", name="sc")
                nc.tensor.matmul(
                    sc[:, 0:QB],
                    km22[0:64, ka * KT : (ka + 1) * KT],
                    q22[0:64, qt * QB : (qt + 1) * QB],
                    start=True,
                    stop=True,
                )
                nc.tensor.matmul(
                    sc[:, QB : 2 * QB],
                    km22[64:128, kb * KT : (kb + 1) * KT],
                    q22[64:128, qt * QB : (qt + 1) * QB],
                    start=True,
                    stop=True,
                )
                pb = PRB.tile([128, 2 * QB], BF16, tag="pb", name="pb")
                nc.scalar.activation(pb[:], sc[:], ACTF.Exp, scale=SCALE)
                nc.tensor.matmul(
                    ctx0[:],
                    vn2[:, ka * 65 : ka * 65 + 65],
                    pb[:, 0:QB],
                    start=(k2 == 0),
                    stop=False,
                )
                nc.tensor.matmul(
                    ctx0[:],
                    vn2[:, kb * 65 : kb * 65 + 65],
                    pb[:, QB : 2 * QB],
                    start=False,
                    stop=(k2 == NKT // 2 - 1),
                )
            epilogue(ctx0, 128, qt)

        # ---- meanV row ----
        for h, vn in ((0, vn0), (1, vn1), (2, vn2)):
            mv = psB.tile([1, 64], F32, tag="cx", name="mv")
            for kt in range(NKT):
                nc.tensor.matmul(
                    mv[:],
                    vn[:, kt * 65 + 64 : kt * 65 + 65],
                    vn[:, kt * 65 : kt * 65 + 64],
                    start=(kt == 0),
                    stop=(kt == NKT - 1),
                )
            nc.vector.tensor_scalar_mul(mvsb[:, h * 64 : (h + 1) * 64], mv[:], 1.0 / S)
        nc.sync.dma_start(t["out"][P_q : P_q + 1, :], mvsb[:])

        # ---- store ----
        for g in range(P_q // 128):
            nc.sync.dma_start(
                t["out"][g * 128 : (g + 1) * 128, :],
                outst[:, g * 192 : (g + 1) * 192],
            )


@functools.lru_cache(maxsize=4)
def _build(P_q):
    nc = bacc.Bacc(
        "TRN2",
        target_bir_lowering=False,
        debug=False,
        enable_asserts=False,
        num_devices=NCORES,
    )
    t = {}
    t["hT_full"] = nc.dram_tensor("hT_full", [HD, S], F32, kind="ExternalInput").ap()
    t["hT_sel"] = nc.dram_tensor("hT_sel", [HD, P_q], F32, kind="ExternalInput").ap()
    for nm in ("q01", "q22", "k01", "k22", "v01", "v22"):
        t[f"w_{nm}"] = nc.dram_tensor(
            f"w_{nm}", [HD, 128], F32, kind="ExternalInput"
        ).ap()
        t[f"b_{nm}"] = nc.dram_tensor(
            f"b_{nm}", [128, 1], F32, kind="ExternalInput"
        ).ap()
    t["ident"] = nc.dram_tensor("ident", [128, 128], F32, kind="ExternalInput").ap()
    t["out"] = nc.dram_tensor("out", [P_q + 1, 192], F32, kind="ExternalOutput").ap()

    with tile.TileContext(nc) as tc:
        _emit(nc, tc, P_q, t)
    nc.compile()
    return nc


def _prep_core_inputs(hidden, sel_pad, Wq, bq, Wk, bk, Wv, bv, heads):
    """Build the in_map for one core. hidden: [S, HD] for this batch."""
    h0, h1, h2 = heads
    m = {}
    m["hT_full"] = np.ascontiguousarray(hidden.T)
    m["hT_sel"] = np.ascontiguousarray(hidden[sel_pad].T)

    def wT(Wmat, h):
        return np.ascontiguousarray(Wmat[h * D : (h + 1) * D, :].T)

    def bs(bvec, h):
        return bvec[h * D : (h + 1) * D]

    for nm, Wmat, bvec in (("q", Wq, bq), ("k", Wk, bk), ("v", Wv, bv)):
        m[f"w_{nm}01"] = np.concatenate([wT(Wmat, h0), wT(Wmat, h1)], axis=1)
        m[f"w_{nm}22"] = np.concatenate([wT(Wmat, h2), wT(Wmat, h2)], axis=1)
        m[f"b_{nm}01"] = np.concatenate([bs(bvec, h0), bs(bvec, h1)]).reshape(128, 1)
        m[f"b_{nm}22"] = np.concatenate([bs(bvec, h2), bs(bvec, h2)]).reshape(128, 1)
    m["ident"] = np.eye(128, dtype=np.float32)
    m = {k: np.ascontiguousarray(v, dtype=np.float32) for k, v in m.items()}
    return m


def _plan(attention_mask):
    """Returns (P_q, sel list, sel_pad list)."""
    sels = [np.where(attention_mask[b] != 0)[0] for b in range(B)]
    nmax = max(1, max(len(s) for s in sels))
    P_q = ((nmax + QB - 1) // QB) * QB
    sel_pads = []
    for s in sels:
        pad = np.zeros(P_q, dtype=np.int64)
        pad[: len(s)] = s
        sel_pads.append(pad)
    return P_q, sels, sel_pads


def build_in_maps(hidden_states, attention_mask, Wq, bq, Wk, bk, Wv, bv):
    P_q, sels, sel_pads = _plan(np.asarray(attention_mask))
    hs = np.asarray(hidden_states, dtype=np.float32)
    in_maps = []
    for c in range(NCORES):
        b, g = c // 4, c % 4
        heads = (3 * g, 3 * g + 1, 3 * g + 2)
        in_maps.append(
            _prep_core_inputs(hs[b], sel_pads[b], Wq, bq, Wk, bk, Wv, bv, heads)
        )
    return P_q, sels, in_maps


def assemble(results, P_q, sels, attention_mask):
    out = np.empty((B, S, HD), dtype=np.float32)
    mask = np.asarray(attention_mask)
    for c in range(NCORES):
        b, g = c // 4, c % 4
        r = results[c]["out"]
        cols = slice(192 * g, 192 * (g + 1))
        sel = sels[b]
        if len(sel):
            out[b, sel, cols] = r[: len(sel)]
        inv = np.where(mask[b] == 0)[0]
        if len(inv):
            out[b, inv, cols] = r[P_q]
    return out


def _install_ntff_shim():
    """Provide antenv.axon_hooks (missing from this image) so
    run_bass_kernel_spmd(trace=True) can capture NTFF profiles, and stub
    out the network-dependent artifact upload."""
    import types

    try:
        import antenv
    except ImportError:
        return
    try:
        from antenv.axon_hooks import get_axon_ntff_profile_hook  # noqa: F401
    except ImportError:
        try:
            if "/root/.axon_site" not in sys.path:
                sys.path.insert(0, "/root/.axon_site")
            from trn_agent_boot.trn_boot import _ntff_profile_via_ctypes

            hook = _ntff_profile_via_ctypes("/opt/axon/libaxon_pjrt.so")
        except Exception:
            hook = None
        mod = types.ModuleType("antenv.axon_hooks")
        _h = {"h": hook}
        mod.get_axon_ntff_profile_hook = lambda: _h["h"]
        mod.set_axon_ntff_profile_hook = lambda h: _h.__setitem__("h", h)
        sys.modules["antenv.axon_hooks"] = mod
        antenv.axon_hooks = mod

    _orig_upload = bass_utils.upload_artifacts

    def _safe_upload(tmpdir):
        try:
            return _orig_upload(tmpdir)
        except Exception:
            return tmpdir

    bass_utils.upload_artifacts = _safe_upload


def kernel(hidden_states, attention_mask, Wq, bq, Wk, bk, Wv, bv, trace=False):
    if trace:
        _install_ntff_shim()
    P_q, sels, in_maps = build_in_maps(
        hidden_states, attention_mask, Wq, bq, Wk, bk, Wv, bv
    )
    nc = _build(P_q)
    res = bass_utils.run_bass_kernel_spmd(
        nc, in_maps, core_ids=list(range(NCORES)), trace=trace
    )
    out = assemble(res.results, P_q, sels, attention_mask)
    if trace:
        kernel.last_exec_time_ns = res.exec_time_ns
        kernel.last_results = res
    return out


# revision 8
# speedup vs baseline: 2.1442x; 1.1477x over previous
"""BertSelfAttention (softsign-modified) Trainium2 Bass kernel.

Sharding: 8 cores = 2 batches x 4 head-groups (3 heads each).
Host gathers unmasked queries (mask applies along the QUERY dim only:
masked rows get uniform softmax => output = mean(V), filled host-side).

Device per core (all fp32):
  - proj: qT/kT/vT = W_hT.T @ hiddenT (hiddenT streamed in slabs)
  - k_mod = k/8 + softsign(softsign(k)/8) + v  (DVE fused ops;
    ss(ss1/8) = ss1/(8+|ss1|))
  - scores^T[k,q] = km^T.T @ qT  (two heads packed in partition halves,
    row-tiled concurrent matmuls)
  - probs = exp(scores/8) on ACT (no max subtraction needed; |s/8|<~8)
  - ctx^T[d,q] + sumexp via lhsT=[V|ones] accumulation over k tiles
  - PE-transpose ctx^T -> ctx, per-partition reciprocal normalize
  - meanV per head appended as the last output row
"""

import functools
import os
import sys

import numpy as np

for _p in ("/opt/trn_rl_repo", "/root/.axon_site/_ro/trn_rl_repo"):
    if os.path.isdir(_p) and _p not in sys.path:
        sys.path.append(_p)

import concourse.bacc as bacc
import concourse.mybir as mybir
import concourse.tile as tile
from concourse import bass_utils

F32 = mybir.dt.float32
BF16 = mybir.dt.float16  # 16-bit matmul dtype (fp16: 10-bit mantissa)
ALU = mybir.AluOpType
ACTF = mybir.ActivationFunctionType

B, S, HD, H, D = 2, 4096, 768, 12, 64
NCORES = 8
HPC = 3  # heads per core
QB = 512  # q block (one PSUM bank of fp32)
KT = 128  # k tile (partition dim of scores^T)
NB = 512  # projection N block
KCH = HD // 128  # 6 contraction chunks
NKT = S // KT  # 32 k tiles
SCALE = 0.125  # 1/sqrt(D)


def _emit(nc, tc, P_q, t):
    """Emit the tile program. t = dict of dram tensor APs."""
    nq = P_q // QB

    with (
        tc.tile_pool(name="persist", bufs=1) as P,
        tc.tile_pool(name="work", bufs=2) as W,
        tc.tile_pool(name="scr", bufs=5) as SCR,
        tc.tile_pool(name="probs", bufs=3) as PRB,
        tc.tile_pool(name="psA", bufs=2, space="PSUM") as psA,
        tc.tile_pool(name="psB", bufs=4, space="PSUM") as psB,
    ):
        # ---- persistent SBUF ----
        q01 = P.tile([128, P_q], BF16)
        q22 = P.tile([128, P_q], BF16)
        k01 = P.tile([128, S], F32)
        k22 = P.tile([128, S], F32)
        km01 = P.tile([128, S], BF16)
        km22 = P.tile([128, S], BF16)
        v01 = P.tile([128, S], F32)
        v22 = P.tile([128, S], F32)
        vn0 = P.tile([128, 65 * NKT], BF16)  # V natural + ones col, head 0
        vn1 = P.tile([128, 65 * NKT], BF16)
        vn2 = P.tile([128, 65 * NKT], BF16)
        outst = P.tile([128, (P_q // 128) * 192], F32)
        ident = P.tile([128, 128], F32)
        mvsb = P.tile([1, 192], F32)

        nc.sync.dma_start(ident[:], t["ident"][:])

        wsb = {}
        bsb = {}
        for nm in ("q01", "q22", "k01", "k22", "v01", "v22"):
            wsb[nm] = P.tile([128, KCH * 128], BF16, name=f"w_{nm}_sb")
            nc.sync.dma_start(
                wsb[nm][:].rearrange("p (c m) -> p c m", c=KCH),
                t[f"w_{nm}"][:].rearrange("(c p) m -> p c m", p=128),
            )
            bsb[nm] = P.tile([128, 1], F32, name=f"b_{nm}_sb")
            nc.sync.dma_start(bsb[nm][:], t[f"b_{nm}"][:])

        def proj_block(src_ap, nb, chains):
            slab = W.tile([128, KCH * NB], BF16, tag="slab", name="slab")
            nc.sync.dma_start(
                slab[:].rearrange("p (c s) -> p c s", c=KCH),
                src_ap[:, nb * NB : (nb + 1) * NB].rearrange("(c p) s -> p c s", p=128),
            )
            for nm, dst in chains:
                ps = psB.tile([128, NB], F32, tag="cx", name="pp")
                for c in range(KCH):
                    nc.tensor.matmul(
                        ps[:],
                        wsb[nm][:, c * 128 : (c + 1) * 128],
                        slab[:, c * NB : (c + 1) * NB],
                        start=(c == 0),
                        stop=(c == KCH - 1),
                    )
                nc.vector.tensor_scalar_add(
                    dst[:, nb * NB : (nb + 1) * NB], ps[:], bsb[nm][:]
                )

        # ---- projections: k/v over full S, q over gathered P_q ----
        for nb in range(S // NB):
            proj_block(
                t["hT_full"],
                nb,
                [("k01", k01), ("k22", k22), ("v01", v01), ("v22", v22)],
            )
        for nb in range(nq):
            proj_block(t["hT_sel"], nb, [("q01", q01), ("q22", q22)])

        # ---- V natural (+ones col) via PE transpose ----
        for vn in (vn0, vn1, vn2):
            nc.gpsimd.memset(vn[:], 1.0)
        for tt in range(NKT):
            pt = psB.tile([128, 128], F32, tag="cx", name="pt")
            nc.tensor.transpose(pt[:], v01[:, tt * 128 : (tt + 1) * 128], ident[:])
            nc.vector.tensor_copy(vn0[:, tt * 65 : tt * 65 + 64], pt[:, 0:64])
            nc.vector.tensor_copy(vn1[:, tt * 65 : tt * 65 + 64], pt[:, 64:128])
            pt2 = psB.tile([128, 128], F32, tag="cx", name="pt2")
            nc.tensor.transpose(pt2[:], v22[:, tt * 128 : (tt + 1) * 128], ident[:])
            nc.vector.tensor_copy(vn2[:, tt * 65 : tt * 65 + 64], pt2[:, 0:64])

        # ---- k_mod (in place into k01/k22) ----
        CH = 1024
        for kbuf, vbuf, kmbuf in ((k01, v01, km01), (k22, v22, km22)):
            for ch in range(S // CH):
                sl = slice(ch * CH, (ch + 1) * CH)
                u = SCR.tile([128, CH], F32, tag="scr", name="u")
                nc.vector.scalar_tensor_tensor(
                    u[:], kbuf[:, sl], SCALE, vbuf[:, sl], op0=ALU.mult, op1=ALU.add
                )
                a1 = SCR.tile([128, CH], F32, tag="scr", name="a1")
                nc.scalar.activation(a1[:], kbuf[:, sl], ACTF.Abs)
                t1 = SCR.tile([128, CH], F32, tag="scr", name="t1")
                nc.vector.tensor_scalar_add(t1[:], a1[:], 1.0)
                r1 = SCR.tile([128, CH], F32, tag="scr", name="r1")
                nc.vector.reciprocal_approx_fast(r1[:], t1[:])
                s1 = SCR.tile([128, CH], F32, tag="scr", name="s1")
                nc.vector.tensor_mul(s1[:], kbuf[:, sl], r1[:])
                a2 = SCR.tile([128, CH], F32, tag="scr", name="a2")
                nc.scalar.activation(a2[:], s1[:], ACTF.Abs)
                t2 = SCR.tile([128, CH], F32, tag="scr", name="t2")
                nc.vector.tensor_scalar_add(t2[:], a2[:], 8.0)
                r2 = SCR.tile([128, CH], F32, tag="scr", name="r2")
                nc.vector.reciprocal_approx_fast(r2[:], t2[:])
                p1 = SCR.tile([128, CH], F32, tag="scr", name="p1")
                nc.vector.tensor_mul(p1[:], s1[:], r2[:])
                nc.vector.tensor_add(kmbuf[:, sl], u[:], p1[:])

        # ---- attention ----
        def epilogue(ctx, col0, qi):
            cs = W.tile([65, QB], F32, tag="cs", name="cs")
            nc.vector.tensor_copy(cs[:], ctx[:])
            for tt in range(QB // 128):
                tp = psB.tile([128, 65], F32, tag="cx", name="tp")
                nc.tensor.transpose(
                    tp[:], cs[:, tt * 128 : (tt + 1) * 128], ident[0:65, 0:65]
                )
                rc = W.tile([128, 1], F32, tag="rc", name="rc")
                nc.vector.reciprocal(rc[:], tp[:, 64:65])
                g = qi * (QB // 128) + tt
                nc.vector.tensor_scalar_mul(
                    outst[:, g * 192 + col0 : g * 192 + col0 + 64], tp[:, 0:64], rc[:]
                )

        def attn_block(kmbuf, qbuf, qa, qb_, vnA, vnB, colA, colB):
            """One (slot0, slot1) pass over all k tiles for q blocks qa/qb_."""
            ctx0 = psB.tile([65, QB], F32, tag="cx", name="ctx0")
            ctx1 = psB.tile([65, QB], F32, tag="cx", name="ctx1")
            for kt in range(NKT):
                sc = psA.tile([128, 2 * QB], F32, tag="sc", name="sc")
                nc.tensor.matmul(
                    sc[:, 0:QB],
                    kmbuf[0:64, kt * KT : (kt + 1) * KT],
                    qbuf[0:64, qa * QB : (qa + 1) * QB],
                    start=True,
                    stop=True,
                )
                nc.tensor.matmul(
                    sc[:, QB : 2 * QB],
                    kmbuf[64:128, kt * KT : (kt + 1) * KT],
                    qbuf[64:128, qb_ * QB : (qb_ + 1) * QB],
                    start=True,
                    stop=True,
                )
                pb = PRB.tile([128, 2 * QB], BF16, tag="pb", name="pb")
                nc.scalar.activation(pb[:], sc[:], ACTF.Exp, scale=SCALE)
                nc.tensor.matmul(
                    ctx0[:],
                    vnA[:, kt * 65 : kt * 65 + 65],
                    pb[:, 0:QB],
                    start=(kt == 0),
                    stop=(kt == NKT - 1),
                )
                nc.tensor.matmul(
                    ctx1[:],
                    vnB[:, kt * 65 : kt * 65 + 65],
                    pb[:, QB : 2 * QB],
                    start=(kt == 0),
                    stop=(kt == NKT - 1),
                )
            epilogue(ctx0, colA, qa)
            epilogue(ctx1, colB, qb_)

        # unit01: heads 0,1 share each q block (slot = head)
        for qi in range(nq):
            attn_block(km01, q01, qi, qi, vn0, vn1, 0, 64)

        # unit22: head 2 self-paired across q blocks (slot = q block)
        for st in range(nq // 2):
            attn_block(km22, q22, 2 * st, 2 * st + 1, vn2, vn2, 128, 128)
        if nq % 2:
            qt = nq - 1
            ctx0 = psB.tile([65, QB], F32, tag="cx", name="ctxT")
            for k2 in range(NKT // 2):
                ka, kb = 2 * k2, 2 * k2 + 1
                sc = psA.tile([128, 2 * QB], F32, tag="sc", name="sc")
                nc.tensor.matmul(
                    sc[:, 0:QB],
                    km22[0:64, ka * KT : (ka + 1) * KT],
                    q22[0:64, qt * QB : (qt + 1) * QB],
                    start=True,
                    stop=True,
                )
                nc.tensor.matmul(
                    sc[:, QB : 2 * QB],
                    km22[64:128, kb * KT : (kb + 1) * KT],
                    q22[64:128, qt * QB : (qt + 1) * QB],
                    start=True,
                    stop=True,
                )
                pb = PRB.tile([128, 2 * QB], BF16, tag="pb", name="pb")
                nc.scalar.activation(pb[:], sc[:], ACTF.Exp, scale=SCALE)
                nc.tensor.matmul(
                    ctx0[:],
                    vn2[:, ka * 65 : ka * 65 + 65],
                    pb[:, 0:QB],
                    start=(k2 == 0),
                    stop=False,
                )
                nc.tensor.matmul(
                    ctx0[:],
                    vn2[:, kb * 65 : kb * 65 + 65],
                    pb[:, QB : 2 * QB],
                    start=False,
                    stop=(k2 == NKT // 2 - 1),
                )
            epilogue(ctx0, 128, qt)

        # ---- meanV row ----
        for h, vn in ((0, vn0), (1, vn1), (2, vn2)):
            mv = psB.tile([1, 64], F32, tag="cx", name="mv")
            for kt in range(NKT):
                nc.tensor.matmul(
                    mv[:],
                    vn[:, kt * 65 + 64 : kt * 65 + 65],
                    vn[:, kt * 65 : kt * 65 + 64],
                    start=(kt == 0),
                    stop=(kt == NKT - 1),
                )
            nc.vector.tensor_scalar_mul(mvsb[:, h * 64 : (h + 1) * 64], mv[:], 1.0 / S)
        nc.sync.dma_start(t["out"][P_q : P_q + 1, :], mvsb[:])

        # ---- store ----
        for g in range(P_q // 128):
            nc.sync.dma_start(
                t["out"][g * 128 : (g + 1) * 128, :],
                outst[:, g * 192 : (g + 1) * 192],
            )


@functools.lru_cache(maxsize=4)
def _build(P_q):
    nc = bacc.Bacc(
        "TRN2",
        target_bir_lowering=False,
        debug=False,
        enable_asserts=False,
        num_devices=NCORES,
    )
    t = {}
    t["hT_full"] = nc.dram_tensor("hT_full", [HD, S], BF16, kind="ExternalInput").ap()
    t["hT_sel"] = nc.dram_tensor("hT_sel", [HD, P_q], BF16, kind="ExternalInput").ap()
    for nm in ("q01", "q22", "k01", "k22", "v01", "v22"):
        t[f"w_{nm}"] = nc.dram_tensor(
            f"w_{nm}", [HD, 128], BF16, kind="ExternalInput"
        ).ap()
        t[f"b_{nm}"] = nc.dram_tensor(
            f"b_{nm}", [128, 1], F32, kind="ExternalInput"
        ).ap()
    t["ident"] = nc.dram_tensor("ident", [128, 128], F32, kind="ExternalInput").ap()
    t["out"] = nc.dram_tensor("out", [P_q + 1, 192], F32, kind="ExternalOutput").ap()

    with tile.TileContext(nc) as tc:
        _emit(nc, tc, P_q, t)
    nc.compile()
    return nc


def _prep_core_inputs(hidden, sel_pad, Wq, bq, Wk, bk, Wv, bv, heads):
    """Build the in_map for one core. hidden: [S, HD] for this batch."""
    h0, h1, h2 = heads
    m = {}
    m["hT_full"] = np.ascontiguousarray(hidden.T.astype(np.float16))
    m["hT_sel"] = np.ascontiguousarray(hidden[sel_pad].T.astype(np.float16))

    def wT(Wmat, h):
        return np.ascontiguousarray(Wmat[h * D : (h + 1) * D, :].T)

    def bs(bvec, h):
        return bvec[h * D : (h + 1) * D]

    for nm, Wmat, bvec in (("q", Wq, bq), ("k", Wk, bk), ("v", Wv, bv)):
        m[f"w_{nm}01"] = np.concatenate([wT(Wmat, h0), wT(Wmat, h1)], axis=1)
        m[f"w_{nm}22"] = np.concatenate([wT(Wmat, h2), wT(Wmat, h2)], axis=1)
        m[f"b_{nm}01"] = np.concatenate([bs(bvec, h0), bs(bvec, h1)]).reshape(128, 1)
        m[f"b_{nm}22"] = np.concatenate([bs(bvec, h2), bs(bvec, h2)]).reshape(128, 1)
    m["ident"] = np.eye(128, dtype=np.float32)
    for k in list(m):
        dt = np.float16 if (k.startswith("w_") or k.startswith("hT_")) else np.float32
        m[k] = np.ascontiguousarray(m[k], dtype=dt)
    return m


def _plan(attention_mask):
    """Returns (P_q, sel list, sel_pad list)."""
    sels = [np.where(attention_mask[b] != 0)[0] for b in range(B)]
    nmax = max(1, max(len(s) for s in sels))
    P_q = ((nmax + QB - 1) // QB) * QB
    sel_pads = []
    for s in sels:
        pad = np.zeros(P_q, dtype=np.int64)
        pad[: len(s)] = s
        sel_pads.append(pad)
    return P_q, sels, sel_pads


def build_in_maps(hidden_states, attention_mask, Wq, bq, Wk, bk, Wv, bv):
    P_q, sels, sel_pads = _plan(np.asarray(attention_mask))
    hs = np.asarray(hidden_states, dtype=np.float32)
    in_maps = []
    for c in range(NCORES):
        b, g = c // 4, c % 4
        heads = (3 * g, 3 * g + 1, 3 * g + 2)
        in_maps.append(
            _prep_core_inputs(hs[b], sel_pads[b], Wq, bq, Wk, bk, Wv, bv, heads)
        )
    return P_q, sels, in_maps


def assemble(results, P_q, sels, attention_mask):
    out = np.empty((B, S, HD), dtype=np.float32)
    mask = np.asarray(attention_mask)
    for c in range(NCORES):
        b, g = c // 4, c % 4
        r = results[c]["out"]
        cols = slice(192 * g, 192 * (g + 1))
        sel = sels[b]
        if len(sel):
            out[b, sel, cols] = r[: len(sel)]
        inv = np.where(mask[b] == 0)[0]
        if len(inv):
            out[b, inv, cols] = r[P_q]
    return out


def _install_ntff_shim():
    """Provide antenv.axon_hooks (missing from this image) so
    run_bass_kernel_spmd(trace=True) can capture NTFF profiles, and stub
    out the network-dependent artifact upload."""
    import types

    try:
        import antenv
    except ImportError:
        return
    try:
        from antenv.axon_hooks import get_axon_ntff_profile_hook  # noqa: F401
    except ImportError:
        try:
            if "/root/.axon_site" not in sys.path:
                sys.path.insert(0, "/root/.axon_site")
            from trn_agent_boot.trn_boot import _ntff_profile_via_ctypes

            hook = _ntff_profile_via_ctypes("/opt/axon/libaxon_pjrt.so")
        except Exception:
            hook = None
        mod = types.ModuleType("antenv.axon_hooks")
        _h = {"h": hook}
        mod.get_axon_ntff_profile_hook = lambda: _h["h"]
        mod.set_axon_ntff_profile_hook = lambda h: _h.__setitem__("h", h)
        sys.modules["antenv.axon_hooks"] = mod
        antenv.axon_hooks = mod

    _orig_upload = bass_utils.upload_artifacts

    def _safe_upload(tmpdir):
        try:
            return _orig_upload(tmpdir)
        except Exception:
            return tmpdir

    bass_utils.upload_artifacts = _safe_upload


def kernel(hidden_states, attention_mask, Wq, bq, Wk, bk, Wv, bv, trace=False):
    if trace:
        _install_ntff_shim()
    P_q, sels, in_maps = build_in_maps(
        hidden_states, attention_mask, Wq, bq, Wk, bk, Wv, bv
    )
    nc = _build(P_q)
    res = bass_utils.run_bass_kernel_spmd(
        nc, in_maps, core_ids=list(range(NCORES)), trace=trace
    )
    out = assemble(res.results, P_q, sels, attention_mask)
    if trace:
        kernel.last_exec_time_ns = res.exec_time_ns
        kernel.last_results = res
    return out


# revision 10
# speedup vs baseline: 2.2089x; 1.0302x over previous
"""BertSelfAttention (softsign-modified) Trainium2 Bass kernel.

Sharding: 8 cores = 2 batches x 4 head-groups (3 heads each).
Host gathers unmasked queries (mask applies along the QUERY dim only:
masked rows get uniform softmax => output = mean(V), filled host-side).

Device per core (all fp32):
  - proj: qT/kT/vT = W_hT.T @ hiddenT (hiddenT streamed in slabs)
  - k_mod = k/8 + softsign(softsign(k)/8) + v  (DVE fused ops;
    ss(ss1/8) = ss1/(8+|ss1|))
  - scores^T[k,q] = km^T.T @ qT  (two heads packed in partition halves,
    row-tiled concurrent matmuls)
  - probs = exp(scores/8) on ACT (no max subtraction needed; |s/8|<~8)
  - ctx^T[d,q] + sumexp via lhsT=[V|ones] accumulation over k tiles
  - PE-transpose ctx^T -> ctx, per-partition reciprocal normalize
  - meanV per head appended as the last output row
"""

import functools
import os
import sys

import numpy as np

for _p in ("/opt/trn_rl_repo", "/root/.axon_site/_ro/trn_rl_repo"):
    if os.path.isdir(_p) and _p not in sys.path:
        sys.path.append(_p)

import concourse.bacc as bacc
import concourse.mybir as mybir
import concourse.tile as tile
from concourse import bass_utils

F32 = mybir.dt.float32
BF16 = mybir.dt.float16  # 16-bit matmul dtype (fp16: 10-bit mantissa)
ALU = mybir.AluOpType
ACTF = mybir.ActivationFunctionType

B, S, HD, H, D = 2, 4096, 768, 12, 64
NCORES = 8
HPC = 3  # heads per core
QB = 512  # q block (one PSUM bank of fp32)
KT = 128  # k tile (partition dim of scores^T)
NB = 512  # projection N block
KCH = HD // 128  # 6 contraction chunks
NKT = S // KT  # 32 k tiles
SCALE = 0.125  # 1/sqrt(D)


def _emit(nc, tc, P_q, t):
    """Emit the tile program. t = dict of dram tensor APs."""
    nq = P_q // QB

    with (
        tc.tile_pool(name="persist", bufs=1) as P,
        tc.tile_pool(name="work", bufs=2) as W,
        tc.tile_pool(name="scr", bufs=5) as SCR,
        tc.tile_pool(name="probs", bufs=3) as PRB,
        tc.tile_pool(name="psA", bufs=2, space="PSUM") as psA,
        tc.tile_pool(name="psB", bufs=4, space="PSUM") as psB,
    ):
        # ---- persistent SBUF ----
        q01 = P.tile([128, P_q], BF16)
        q22 = P.tile([128, P_q], BF16)
        k01 = P.tile([128, S], F32)
        k22 = P.tile([128, S], F32)
        km01 = P.tile([128, S], BF16)
        km22 = P.tile([128, S], BF16)
        v01 = P.tile([128, S], F32)
        v22 = P.tile([128, S], F32)
        vn0 = P.tile([128, 65 * NKT], BF16)  # V natural + ones col, head 0
        vn1 = P.tile([128, 65 * NKT], BF16)
        vn2 = P.tile([128, 65 * NKT], BF16)
        outst = P.tile([128, (P_q // 128) * 192], F32)
        ident = P.tile([128, 128], F32)
        mvsb = P.tile([1, 192], F32)

        nc.sync.dma_start(ident[:], t["ident"][:])

        wsb = {}
        bsb = {}
        for nm in ("q01", "q22", "k01", "k22", "v01", "v22"):
            wsb[nm] = P.tile([128, KCH * 128], BF16, name=f"w_{nm}_sb")
            nc.sync.dma_start(
                wsb[nm][:].rearrange("p (c m) -> p c m", c=KCH),
                t[f"w_{nm}"][:].rearrange("(c p) m -> p c m", p=128),
            )
            bsb[nm] = P.tile([128, 1], F32, name=f"b_{nm}_sb")
            nc.sync.dma_start(bsb[nm][:], t[f"b_{nm}"][:])

        def proj_block(src_ap, nb, chains):
            slab = W.tile([128, KCH * NB], BF16, tag="slab", name="slab")
            nc.sync.dma_start(
                slab[:].rearrange("p (c s) -> p c s", c=KCH),
                src_ap[:, nb * NB : (nb + 1) * NB].rearrange("(c p) s -> p c s", p=128),
            )
            for nm, dst in chains:
                ps = psB.tile([128, NB], F32, tag="cx", name="pp")
                for c in range(KCH):
                    nc.tensor.matmul(
                        ps[:],
                        wsb[nm][:, c * 128 : (c + 1) * 128],
                        slab[:, c * NB : (c + 1) * NB],
                        start=(c == 0),
                        stop=(c == KCH - 1),
                    )
                nc.vector.tensor_scalar_add(
                    dst[:, nb * NB : (nb + 1) * NB], ps[:], bsb[nm][:]
                )

        # ---- per-unit ingredient emitters ----
        def emit_vnat(vbuf, dsts):
            """PE-transpose vbuf 128-col slices into natural layout; dsts is
            a list of (vn, src_col0) pairs taking out[:, src_col0:+64]."""
            for tt in range(NKT):
                pt = psB.tile([128, 128], F32, tag="cx", name="pt")
                nc.tensor.transpose(pt[:], vbuf[:, tt * 128 : (tt + 1) * 128], ident[:])
                for vn, c0 in dsts:
                    nc.vector.tensor_copy(
                        vn[:, tt * 65 : tt * 65 + 64], pt[:, c0 : c0 + 64]
                    )

        CH = 1024

        def emit_km(kbuf, vbuf, kmbuf):
            for ch in range(S // CH):
                sl = slice(ch * CH, (ch + 1) * CH)
                u = SCR.tile([128, CH], F32, tag="scr", name="u")
                nc.vector.scalar_tensor_tensor(
                    u[:], kbuf[:, sl], SCALE, vbuf[:, sl], op0=ALU.mult, op1=ALU.add
                )
                a1 = SCR.tile([128, CH], F32, tag="scr", name="a1")
                nc.scalar.activation(a1[:], kbuf[:, sl], ACTF.Abs)
                t1 = SCR.tile([128, CH], F32, tag="scr", name="t1")
                nc.vector.tensor_scalar_add(t1[:], a1[:], 1.0)
                r1 = SCR.tile([128, CH], F32, tag="scr", name="r1")
                nc.vector.reciprocal_approx_fast(r1[:], t1[:])
                s1 = SCR.tile([128, CH], F32, tag="scr", name="s1")
                nc.vector.tensor_mul(s1[:], kbuf[:, sl], r1[:])
                a2 = SCR.tile([128, CH], F32, tag="scr", name="a2")
                nc.scalar.activation(a2[:], s1[:], ACTF.Abs)
                t2 = SCR.tile([128, CH], F32, tag="scr", name="t2")
                nc.vector.tensor_scalar_add(t2[:], a2[:], 8.0)
                r2 = SCR.tile([128, CH], F32, tag="scr", name="r2")
                nc.vector.reciprocal_approx_fast(r2[:], t2[:])
                p1 = SCR.tile([128, CH], F32, tag="scr", name="p1")
                nc.vector.tensor_mul(p1[:], s1[:], r2[:])
                nc.vector.tensor_add(kmbuf[:, sl], u[:], p1[:])

        # ---- attention ----
        def epilogue(ctx, col0, qi):
            cs = W.tile([65, QB], F32, tag="cs", name="cs")
            nc.vector.tensor_copy(cs[:], ctx[:])
            for tt in range(QB // 128):
                tp = psB.tile([128, 65], F32, tag="cx", name="tp")
                nc.tensor.transpose(
                    tp[:], cs[:, tt * 128 : (tt + 1) * 128], ident[0:65, 0:65]
                )
                rc = W.tile([128, 1], F32, tag="rc", name="rc")
                nc.vector.reciprocal(rc[:], tp[:, 64:65])
                g = qi * (QB // 128) + tt
                nc.vector.tensor_scalar_mul(
                    outst[:, g * 192 + col0 : g * 192 + col0 + 64], tp[:, 0:64], rc[:]
                )

        def attn_block(kmbuf, qbuf, qa, qb_, vnA, vnB, colA, colB):
            """One (slot0, slot1) pass over all k tiles for q blocks qa/qb_."""
            ctx0 = psB.tile([65, QB], F32, tag="cx", name="ctx0")
            ctx1 = psB.tile([65, QB], F32, tag="cx", name="ctx1")
            for kt in range(NKT):
                sc = psA.tile([128, 2 * QB], F32, tag="sc", name="sc")
                nc.tensor.matmul(
                    sc[:, 0:QB],
                    kmbuf[0:64, kt * KT : (kt + 1) * KT],
                    qbuf[0:64, qa * QB : (qa + 1) * QB],
                    start=True,
                    stop=True,
                )
                nc.tensor.matmul(
                    sc[:, QB : 2 * QB],
                    kmbuf[64:128, kt * KT : (kt + 1) * KT],
                    qbuf[64:128, qb_ * QB : (qb_ + 1) * QB],
                    start=True,
                    stop=True,
                )
                pb = PRB.tile([128, 2 * QB], BF16, tag="pb", name="pb")
                nc.scalar.activation(pb[:], sc[:], ACTF.Exp, scale=SCALE)
                nc.tensor.matmul(
                    ctx0[:],
                    vnA[:, kt * 65 : kt * 65 + 65],
                    pb[:, 0:QB],
                    start=(kt == 0),
                    stop=(kt == NKT - 1),
                )
                nc.tensor.matmul(
                    ctx1[:],
                    vnB[:, kt * 65 : kt * 65 + 65],
                    pb[:, QB : 2 * QB],
                    start=(kt == 0),
                    stop=(kt == NKT - 1),
                )
            epilogue(ctx0, colA, qa)
            epilogue(ctx1, colB, qb_)

        # ---- unit01 ingredients, then its attention; unit22's projection
        # work is emitted after so the scheduler overlaps it with unit01's
        # (ACT-bound) attention ----
        for vn in (vn0, vn1, vn2):
            nc.gpsimd.memset(vn[:], 1.0)

        for nb in range(S // NB):
            proj_block(t["hT_full"], nb, [("k01", k01), ("v01", v01)])
        emit_km(k01, v01, km01)
        emit_vnat(v01, [(vn0, 0), (vn1, 64)])
        for nb in range(nq):
            proj_block(t["hT_sel"], nb, [("q01", q01)])

        # unit01: heads 0,1 share each q block (slot = head)
        for qi in range(nq):
            attn_block(km01, q01, qi, qi, vn0, vn1, 0, 64)

        # unit22 ingredients (overlap with unit01 attention)
        for nb in range(S // NB):
            proj_block(t["hT_full"], nb, [("k22", k22), ("v22", v22)])
        emit_km(k22, v22, km22)
        emit_vnat(v22, [(vn2, 0)])
        for nb in range(nq):
            proj_block(t["hT_sel"], nb, [("q22", q22)])

        # unit22: head 2 self-paired across q blocks (slot = q block)
        for st in range(nq // 2):
            attn_block(km22, q22, 2 * st, 2 * st + 1, vn2, vn2, 128, 128)
        if nq % 2:
            qt = nq - 1
            ctx0 = psB.tile([65, QB], F32, tag="cx", name="ctxT")
            for k2 in range(NKT // 2):
                ka, kb = 2 * k2, 2 * k2 + 1
                sc = psA.tile([128, 2 * QB], F32, tag="sc", name="sc")
                nc.tensor.matmul(
                    sc[:, 0:QB],
                    km22[0:64, ka * KT : (ka + 1) * KT],
                    q22[0:64, qt * QB : (qt + 1) * QB],
                    start=True,
                    stop=True,
                )
                nc.tensor.matmul(
                    sc[:, QB : 2 * QB],
                    km22[64:128, kb * KT : (kb + 1) * KT],
                    q22[64:128, qt * QB : (qt + 1) * QB],
                    start=True,
                    stop=True,
                )
                pb = PRB.tile([128, 2 * QB], BF16, tag="pb", name="pb")
                nc.scalar.activation(pb[:], sc[:], ACTF.Exp, scale=SCALE)
                nc.tensor.matmul(
                    ctx0[:],
                    vn2[:, ka * 65 : ka * 65 + 65],
                    pb[:, 0:QB],
                    start=(k2 == 0),
                    stop=False,
                )
                nc.tensor.matmul(
                    ctx0[:],
                    vn2[:, kb * 65 : kb * 65 + 65],
                    pb[:, QB : 2 * QB],
                    start=False,
                    stop=(k2 == NKT // 2 - 1),
                )
            epilogue(ctx0, 128, qt)

        # ---- meanV row ----
        for h, vn in ((0, vn0), (1, vn1), (2, vn2)):
            mv = psB.tile([1, 64], F32, tag="cx", name="mv")
            for kt in range(NKT):
                nc.tensor.matmul(
                    mv[:],
                    vn[:, kt * 65 + 64 : kt * 65 + 65],
                    vn[:, kt * 65 : kt * 65 + 64],
                    start=(kt == 0),
                    stop=(kt == NKT - 1),
                )
            nc.vector.tensor_scalar_mul(mvsb[:, h * 64 : (h + 1) * 64], mv[:], 1.0 / S)
        nc.sync.dma_start(t["out"][P_q : P_q + 1, :], mvsb[:])

        # ---- store ----
        for g in range(P_q // 128):
            nc.sync.dma_start(
                t["out"][g * 128 : (g + 1) * 128, :],
                outst[:, g * 192 : (g + 1) * 192],
            )


@functools.lru_cache(maxsize=4)
def _build(P_q):
    nc = bacc.Bacc(
        "TRN2",
        target_bir_lowering=False,
        debug=False,
        enable_asserts=False,
        num_devices=NCORES,
    )
    t = {}
    t["hT_full"] = nc.dram_tensor("hT_full", [HD, S], BF16, kind="ExternalInput").ap()
    t["hT_sel"] = nc.dram_tensor("hT_sel", [HD, P_q], BF16, kind="ExternalInput").ap()
    for nm in ("q01", "q22", "k01", "k22", "v01", "v22"):
        t[f"w_{nm}"] = nc.dram_tensor(
            f"w_{nm}", [HD, 128], BF16, kind="ExternalInput"
        ).ap()
        t[f"b_{nm}"] = nc.dram_tensor(
            f"b_{nm}", [128, 1], F32, kind="ExternalInput"
        ).ap()
    t["ident"] = nc.dram_tensor("ident", [128, 128], F32, kind="ExternalInput").ap()
    t["out"] = nc.dram_tensor("out", [P_q + 1, 192], F32, kind="ExternalOutput").ap()

    with tile.TileContext(nc) as tc:
        _emit(nc, tc, P_q, t)
    nc.compile()
    return nc


def _prep_core_inputs(hidden, sel_pad, Wq, bq, Wk, bk, Wv, bv, heads):
    """Build the in_map for one core. hidden: [S, HD] for this batch."""
    h0, h1, h2 = heads
    m = {}
    m["hT_full"] = np.ascontiguousarray(hidden.T.astype(np.float16))
    m["hT_sel"] = np.ascontiguousarray(hidden[sel_pad].T.astype(np.float16))

    def wT(Wmat, h):
        return np.ascontiguousarray(Wmat[h * D : (h + 1) * D, :].T)

    def bs(bvec, h):
        return bvec[h * D : (h + 1) * D]

    for nm, Wmat, bvec in (("q", Wq, bq), ("k", Wk, bk), ("v", Wv, bv)):
        m[f"w_{nm}01"] = np.concatenate([wT(Wmat, h0), wT(Wmat, h1)], axis=1)
        m[f"w_{nm}22"] = np.concatenate([wT(Wmat, h2), wT(Wmat, h2)], axis=1)
        m[f"b_{nm}01"] = np.concatenate([bs(bvec, h0), bs(bvec, h1)]).reshape(128, 1)
        m[f"b_{nm}22"] = np.concatenate([bs(bvec, h2), bs(bvec, h2)]).reshape(128, 1)
    m["ident"] = np.eye(128, dtype=np.float32)
    for k in list(m):
        dt = np.float16 if (k.startswith("w_") or k.startswith("hT_")) else np.float32
        m[k] = np.ascontiguousarray(m[k], dtype=dt)
    return m


def _plan(attention_mask):
    """Returns (P_q, sel list, sel_pad list)."""
    sels = [np.where(attention_mask[b] != 0)[0] for b in range(B)]
    nmax = max(1, max(len(s) for s in sels))
    P_q = ((nmax + QB - 1) // QB) * QB
    sel_pads = []
    for s in sels:
        pad = np.zeros(P_q, dtype=np.int64)
        pad[: len(s)] = s
        sel_pads.append(pad)
    return P_q, sels, sel_pads


def build_in_maps(hidden_states, attention_mask, Wq, bq, Wk, bk, Wv, bv):
    P_q, sels, sel_pads = _plan(np.asarray(attention_mask))
    hs = np.asarray(hidden_states, dtype=np.float32)
    in_maps = []
    for c in range(NCORES):
        b, g = c // 4, c % 4
        heads = (3 * g, 3 * g + 1, 3 * g + 2)
        in_maps.append(
            _prep_core_inputs(hs[b], sel_pads[b], Wq, bq, Wk, bk, Wv, bv, heads)
        )
    return P_q, sels, in_maps


def assemble(results, P_q, sels, attention_mask):
    out = np.empty((B, S, HD), dtype=np.float32)
    mask = np.asarray(attention_mask)
    for c in range(NCORES):
        b, g = c // 4, c % 4
        r = results[c]["out"]
        cols = slice(192 * g, 192 * (g + 1))
        sel = sels[b]
        if len(sel):
            out[b, sel, cols] = r[: len(sel)]
        inv = np.where(mask[b] == 0)[0]
        if len(inv):
            out[b, inv, cols] = r[P_q]
    return out


def _install_ntff_shim():
    """Provide antenv.axon_hooks (missing from this image) so
    run_bass_kernel_spmd(trace=True) can capture NTFF profiles, and stub
    out the network-dependent artifact upload."""
    import types

    try:
        import antenv
    except ImportError:
        return
    try:
        from antenv.axon_hooks import get_axon_ntff_profile_hook  # noqa: F401
    except ImportError:
        try:
            if "/root/.axon_site" not in sys.path:
                sys.path.insert(0, "/root/.axon_site")
            from trn_agent_boot.trn_boot import _ntff_profile_via_ctypes

            hook = _ntff_profile_via_ctypes("/opt/axon/libaxon_pjrt.so")
        except Exception:
            hook = None
        mod = types.ModuleType("antenv.axon_hooks")
        _h = {"h": hook}
        mod.get_axon_ntff_profile_hook = lambda: _h["h"]
        mod.set_axon_ntff_profile_hook = lambda h: _h.__setitem__("h", h)
        sys.modules["antenv.axon_hooks"] = mod
        antenv.axon_hooks = mod

    _orig_upload = bass_utils.upload_artifacts

    def _safe_upload(tmpdir):
        try:
            return _orig_upload(tmpdir)
        except Exception:
            return tmpdir

    bass_utils.upload_artifacts = _safe_upload


def kernel(hidden_states, attention_mask, Wq, bq, Wk, bk, Wv, bv, trace=False):
    if trace:
        _install_ntff_shim()
    P_q, sels, in_maps = build_in_maps(
        hidden_states, attention_mask, Wq, bq, Wk, bk, Wv, bv
    )
    nc = _build(P_q)
    res = bass_utils.run_bass_kernel_spmd(
        nc, in_maps, core_ids=list(range(NCORES)), trace=trace
    )
    out = assemble(res.results, P_q, sels, attention_mask)
    if trace:
        kernel.last_exec_time_ns = res.exec_time_ns
        kernel.last_results = res
    return out


# revision 15
# speedup vs baseline: 2.3389x; 1.0589x over previous
"""BertSelfAttention (softsign-modified) Trainium2 Bass kernel.

Sharding: 8 cores = 2 batches x 4 head-groups (3 heads each).
Host gathers unmasked queries (mask applies along the QUERY dim only:
masked rows get uniform softmax => output = mean(V), filled host-side).

Device per core (all fp32):
  - proj: qT/kT/vT = W_hT.T @ hiddenT (hiddenT streamed in slabs)
  - k_mod = k/8 + softsign(softsign(k)/8) + v  (DVE fused ops;
    ss(ss1/8) = ss1/(8+|ss1|))
  - scores^T[k,q] = km^T.T @ qT  (two heads packed in partition halves,
    row-tiled concurrent matmuls)
  - probs = exp(scores/8) on ACT (no max subtraction needed; |s/8|<~8)
  - ctx^T[d,q] + sumexp via lhsT=[V|ones] accumulation over k tiles
  - PE-transpose ctx^T -> ctx, per-partition reciprocal normalize
  - meanV per head appended as the last output row
"""

import functools
import os
import sys

import numpy as np

for _p in ("/opt/trn_rl_repo", "/root/.axon_site/_ro/trn_rl_repo"):
    if os.path.isdir(_p) and _p not in sys.path:
        sys.path.append(_p)

import concourse.bacc as bacc
import concourse.mybir as mybir
import concourse.tile as tile
from concourse import bass_utils

F32 = mybir.dt.float32
BF16 = mybir.dt.float16  # 16-bit matmul dtype (fp16: 10-bit mantissa)
ALU = mybir.AluOpType
ACTF = mybir.ActivationFunctionType

B, S, HD, H, D = 2, 4096, 768, 12, 64
NCORES = 8
HPC = 3  # heads per core
QB = 512  # q block (one PSUM bank of fp32)
KT = 128  # k tile (partition dim of scores^T)
NB = 512  # projection N block
KCH = HD // 128  # 6 contraction chunks
NKT = S // KT  # 32 k tiles
SCALE = 0.125  # 1/sqrt(D)


def _qblocks(P_q):
    """Split P_q into blocks: 512s then one optional 256 tail."""
    out = []
    q0 = 0
    while P_q - q0 >= QB:
        out.append((q0, QB))
        q0 += QB
    if P_q - q0:
        out.append((q0, P_q - q0))
    return out


def _emit(nc, tc, P_q, t):
    """Emit the tile program. t = dict of dram tensor APs."""
    qbs = _qblocks(P_q)

    with (
        tc.tile_pool(name="persist", bufs=1) as P,
        tc.tile_pool(name="work", bufs=2) as W,
        tc.tile_pool(name="scr", bufs=5) as SCR,
        tc.tile_pool(name="probs", bufs=3) as PRB,
        tc.tile_pool(name="psA", bufs=2, space="PSUM") as psA,
        tc.tile_pool(name="psB", bufs=4, space="PSUM") as psB,
    ):
        # ---- persistent SBUF ----
        q01 = P.tile([128, P_q], BF16)
        q22 = P.tile([128, P_q], BF16)
        k01 = P.tile([128, S], F32)
        k22 = P.tile([128, S], F32)
        km01 = P.tile([128, S], BF16)
        km22 = P.tile([128, S], BF16)
        v01 = P.tile([128, S], F32)
        v22 = P.tile([128, S], F32)
        vn0 = P.tile([128, 65 * NKT], BF16)  # V natural + ones col, head 0
        vn1 = P.tile([128, 65 * NKT], BF16)
        vn2 = P.tile([128, 65 * NKT], BF16)
        outst = P.tile([128, (P_q // 128) * 192], F32)
        ident = P.tile([128, 128], F32)
        mvsb = P.tile([1, 192], F32)

        nc.sync.dma_start(ident[:], t["ident"][:])

        wsb = {}
        bsb = {}
        for nm in ("q01", "q22", "k01", "k22", "v01", "v22"):
            wsb[nm] = P.tile([128, KCH * 128], BF16, name=f"w_{nm}_sb")
            nc.sync.dma_start(
                wsb[nm][:].rearrange("p (c m) -> p c m", c=KCH),
                t[f"w_{nm}"][:].rearrange("(c p) m -> p c m", p=128),
            )
            bsb[nm] = P.tile([128, 1], F32, name=f"b_{nm}_sb")
            nc.sync.dma_start(bsb[nm][:], t[f"b_{nm}"][:])

        def proj_block(src_ap, blk, chains, on_act):
            n0, w = blk
            slab = W.tile([128, KCH * NB], BF16, tag="slab", name="slab")
            nc.sync.dma_start(
                slab[:, 0 : KCH * w].rearrange("p (c s) -> p c s", c=KCH),
                src_ap[:, n0 : n0 + w].rearrange("(c p) s -> p c s", p=128),
            )
            for nm, dst in chains:
                ps = psB.tile([128, NB], F32, tag="cx", name="pp")
                for c in range(KCH):
                    nc.tensor.matmul(
                        ps[:, 0:w],
                        wsb[nm][:, c * 128 : (c + 1) * 128],
                        slab[:, c * w : (c + 1) * w],
                        start=(c == 0),
                        stop=(c == KCH - 1),
                    )
                dsl = dst[:, n0 : n0 + w]
                if on_act:
                    nc.scalar.activation(
                        dsl, ps[:, 0:w], ACTF.Identity, bias=bsb[nm][:]
                    )
                else:
                    nc.vector.tensor_scalar_add(dsl, ps[:, 0:w], bsb[nm][:])

        def emit_vnat(vbuf, dsts, tts, on_act):
            for tt in tts:
                pt = psB.tile([128, 128], F32, tag="cx", name="pt")
                nc.tensor.transpose(pt[:], vbuf[:, tt * 128 : (tt + 1) * 128], ident[:])
                for vn, c0 in dsts:
                    dsl = vn[:, tt * 65 : tt * 65 + 64]
                    if on_act:
                        nc.scalar.copy(dsl, pt[:, c0 : c0 + 64])
                    else:
                        nc.vector.tensor_copy(dsl, pt[:, c0 : c0 + 64])

        CH = 1024
        U32 = mybir.dt.uint32

        def emit_km_chunk(kbuf, vbuf, kmbuf, ch, abs_on_act):
            sl = slice(ch * CH, (ch + 1) * CH)
            u = SCR.tile([128, CH], F32, tag="scr", name="u")
            nc.vector.scalar_tensor_tensor(
                u[:], kbuf[:, sl], SCALE, vbuf[:, sl], op0=ALU.mult, op1=ALU.add
            )
            a1 = SCR.tile([128, CH], F32, tag="scr", name="a1")
            if abs_on_act:
                nc.scalar.activation(a1[:], kbuf[:, sl], ACTF.Abs)
            else:
                nc.vector.tensor_scalar(
                    a1[:].bitcast(U32), kbuf[:, sl].bitcast(U32),
                    0x7FFFFFFF, None, op0=ALU.bitwise_and,
                )
            t1 = SCR.tile([128, CH], F32, tag="scr", name="t1")
            nc.vector.tensor_scalar_add(t1[:], a1[:], 1.0)
            r1 = SCR.tile([128, CH], F32, tag="scr", name="r1")
            nc.vector.reciprocal_approx_fast(r1[:], t1[:])
            s1 = SCR.tile([128, CH], F32, tag="scr", name="s1")
            nc.vector.tensor_mul(s1[:], kbuf[:, sl], r1[:])
            a2 = SCR.tile([128, CH], F32, tag="scr", name="a2")
            if abs_on_act:
                nc.scalar.activation(a2[:], s1[:], ACTF.Abs)
            else:
                nc.vector.tensor_scalar(
                    a2[:].bitcast(U32), s1[:].bitcast(U32),
                    0x7FFFFFFF, None, op0=ALU.bitwise_and,
                )
            t2 = SCR.tile([128, CH], F32, tag="scr", name="t2")
            nc.vector.tensor_scalar_add(t2[:], a2[:], 8.0)
            r2 = SCR.tile([128, CH], F32, tag="scr", name="r2")
            nc.vector.reciprocal_approx_fast(r2[:], t2[:])
            p1 = SCR.tile([128, CH], F32, tag="scr", name="p1")
            nc.vector.tensor_mul(p1[:], s1[:], r2[:])
            nc.vector.tensor_add(kmbuf[:, sl], u[:], p1[:])

        # ---- attention ----
        def epilogue(ctx, w, col0, q0):
            cs = W.tile([65, QB], F32, tag="cs", name="cs")
            nc.vector.tensor_copy(cs[:, 0:w], ctx[:, 0:w])
            for tt in range(w // 128):
                tp = psB.tile([128, 65], F32, tag="cx", name="tp")
                nc.tensor.transpose(
                    tp[:], cs[:, tt * 128 : (tt + 1) * 128], ident[0:65, 0:65]
                )
                rc = W.tile([128, 1], F32, tag="rc", name="rc")
                nc.vector.reciprocal(rc[:], tp[:, 64:65])
                g = q0 // 128 + tt
                nc.vector.tensor_scalar_mul(
                    outst[:, g * 192 + col0 : g * 192 + col0 + 64], tp[:, 0:64], rc[:]
                )

        def attn_block(kmbuf, qbuf, blkA, blkB, vnA, vnB, colA, colB):
            """One (slot0, slot1) pass over all k tiles; blk = (q0, width)."""
            qa, wa = blkA
            qb_, wb = blkB
            ctx0 = psB.tile([65, QB], F32, tag="cx", name="ctx0")
            ctx1 = psB.tile([65, QB], F32, tag="cx", name="ctx1")
            for kt in range(NKT):
                sc = psA.tile([128, 2 * QB], F32, tag="sc", name="sc")
                nc.tensor.matmul(
                    sc[:, 0:wa],
                    kmbuf[0:64, kt * KT : (kt + 1) * KT],
                    qbuf[0:64, qa : qa + wa],
                    start=True,
                    stop=True,
                )
                nc.tensor.matmul(
                    sc[:, QB : QB + wb],
                    kmbuf[64:128, kt * KT : (kt + 1) * KT],
                    qbuf[64:128, qb_ : qb_ + wb],
                    start=True,
                    stop=True,
                )
                pb = PRB.tile([128, 2 * QB], BF16, tag="pb", name="pb")
                if wa == QB:
                    nc.scalar.activation(pb[:, 0 : QB + wb], sc[:, 0 : QB + wb],
                                         ACTF.Exp, scale=SCALE)
                else:
                    nc.scalar.activation(pb[:, 0:wa], sc[:, 0:wa],
                                         ACTF.Exp, scale=SCALE)
                    nc.scalar.activation(pb[:, QB : QB + wb], sc[:, QB : QB + wb],
                                         ACTF.Exp, scale=SCALE)
                nc.tensor.matmul(
                    ctx0[:, 0:wa],
                    vnA[:, kt * 65 : kt * 65 + 65],
                    pb[:, 0:wa],
                    start=(kt == 0),
                    stop=(kt == NKT - 1),
                )
                nc.tensor.matmul(
                    ctx1[:, 0:wb],
                    vnB[:, kt * 65 : kt * 65 + 65],
                    pb[:, QB : QB + wb],
                    start=(kt == 0),
                    stop=(kt == NKT - 1),
                )
            epilogue(ctx0, wa, colA, qa)
            epilogue(ctx1, wb, colB, qb_)

        def attn_tail(kmbuf, qbuf, blk, vn, col0):
            """Single q block, k tiles processed in row-tiled pairs."""
            qt, wt = blk
            ctx0 = psB.tile([65, QB], F32, tag="cx", name="ctxT")
            for k2 in range(NKT // 2):
                ka, kb = 2 * k2, 2 * k2 + 1
                sc = psA.tile([128, 2 * QB], F32, tag="sc", name="sc")
                nc.tensor.matmul(
                    sc[:, 0:wt],
                    kmbuf[0:64, ka * KT : (ka + 1) * KT],
                    qbuf[0:64, qt : qt + wt],
                    start=True,
                    stop=True,
                )
                nc.tensor.matmul(
                    sc[:, QB : QB + wt],
                    kmbuf[64:128, kb * KT : (kb + 1) * KT],
                    qbuf[64:128, qt : qt + wt],
                    start=True,
                    stop=True,
                )
                pb = PRB.tile([128, 2 * QB], BF16, tag="pb", name="pb")
                if wt == QB:
                    nc.scalar.activation(pb[:, 0 : 2 * QB], sc[:, 0 : 2 * QB],
                                         ACTF.Exp, scale=SCALE)
                else:
                    nc.scalar.activation(pb[:, 0:wt], sc[:, 0:wt],
                                         ACTF.Exp, scale=SCALE)
                    nc.scalar.activation(pb[:, QB : QB + wt], sc[:, QB : QB + wt],
                                         ACTF.Exp, scale=SCALE)
                nc.tensor.matmul(
                    ctx0[:, 0:wt],
                    vn[:, ka * 65 : ka * 65 + 65],
                    pb[:, 0:wt],
                    start=(k2 == 0),
                    stop=False,
                )
                nc.tensor.matmul(
                    ctx0[:, 0:wt],
                    vn[:, kb * 65 : kb * 65 + 65],
                    pb[:, QB : QB + wt],
                    start=False,
                    stop=(k2 == NKT // 2 - 1),
                )
            epilogue(ctx0, wt, col0, qt)

        # ---- unit01 ingredients (ramp: bias/copies on ACT, abs on ACT) ----
        for vn in (vn0, vn1, vn2):
            nc.gpsimd.memset(vn[:], 1.0)

        for nb in range(S // NB):
            proj_block(
                t["hT_full"], (nb * NB, NB), [("k01", k01), ("v01", v01)], on_act=True
            )
        for ch in range(S // CH):
            emit_km_chunk(k01, v01, km01, ch, abs_on_act=True)
        emit_vnat(v01, [(vn0, 0), (vn1, 64)], range(NKT), on_act=True)
        for blk in qbs:
            proj_block(t["hT_sel"], blk, [("q01", q01)], on_act=True)

        # ---- unit22 ingredient slices, interleaved into unit01 attention
        # (ACT is saturated by exp there, so these use DVE/PE spare) ----
        u22_slices = []
        for i in range(4):
            sl = []
            sl.append(("kv", 2 * i))
            sl.append(("kv", 2 * i + 1))
            sl.append(("km", i))
            sl.append(("vn", i))
            u22_slices.append(sl)
        for i, blk in enumerate(qbs):
            u22_slices[i % 4].append(("q", blk))

        def emit_u22_slice(sl):
            for kind, i in sl:
                if kind == "kv":
                    proj_block(
                        t["hT_full"], (i * NB, NB), [("k22", k22), ("v22", v22)],
                        on_act=False,
                    )
                elif kind == "km":
                    emit_km_chunk(k22, v22, km22, i, abs_on_act=False)
                elif kind == "vn":
                    emit_vnat(v22, [(vn2, 0)], range(8 * i, 8 * i + 8), on_act=False)
                else:
                    proj_block(t["hT_sel"], i, [("q22", q22)], on_act=False)


        # unit01 attention with u22 ingredients interleaved
        for qi, blk in enumerate(qbs):
            attn_block(km01, q01, blk, blk, vn0, vn1, 0, 64)
            if qi < len(u22_slices):
                emit_u22_slice(u22_slices[qi])
        for sl in u22_slices[len(qbs) :]:
            emit_u22_slice(sl)

        # unit22: head 2 self-paired across q blocks
        for st in range(len(qbs) // 2):
            attn_block(
                km22, q22, qbs[2 * st], qbs[2 * st + 1], vn2, vn2, 128, 128
            )
        if len(qbs) % 2:
            attn_tail(km22, q22, qbs[-1], vn2, 128)

        # ---- meanV row ----
        for h, vn in ((0, vn0), (1, vn1), (2, vn2)):
            mv = psB.tile([1, 64], F32, tag="cx", name="mv")
            for kt in range(NKT):
                nc.tensor.matmul(
                    mv[:],
                    vn[:, kt * 65 + 64 : kt * 65 + 65],
                    vn[:, kt * 65 : kt * 65 + 64],
                    start=(kt == 0),
                    stop=(kt == NKT - 1),
                )
            nc.vector.tensor_scalar_mul(mvsb[:, h * 64 : (h + 1) * 64], mv[:], 1.0 / S)
        nc.sync.dma_start(t["out"][P_q : P_q + 1, :], mvsb[:])

        # ---- store ----
        for g in range(P_q // 128):
            nc.sync.dma_start(
                t["out"][g * 128 : (g + 1) * 128, :],
                outst[:, g * 192 : (g + 1) * 192],
            )


@functools.lru_cache(maxsize=4)
def _build(P_q):
    nc = bacc.Bacc(
        "TRN2",
        target_bir_lowering=False,
        debug=False,
        enable_asserts=False,
        num_devices=NCORES,
    )
    t = {}
    t["hT_full"] = nc.dram_tensor("hT_full", [HD, S], BF16, kind="ExternalInput").ap()
    t["hT_sel"] = nc.dram_tensor("hT_sel", [HD, P_q], BF16, kind="ExternalInput").ap()
    for nm in ("q01", "q22", "k01", "k22", "v01", "v22"):
        t[f"w_{nm}"] = nc.dram_tensor(
            f"w_{nm}", [HD, 128], BF16, kind="ExternalInput"
        ).ap()
        t[f"b_{nm}"] = nc.dram_tensor(
            f"b_{nm}", [128, 1], F32, kind="ExternalInput"
        ).ap()
    t["ident"] = nc.dram_tensor("ident", [128, 128], F32, kind="ExternalInput").ap()
    t["out"] = nc.dram_tensor("out", [P_q + 1, 192], F32, kind="ExternalOutput").ap()

    with tile.TileContext(nc) as tc:
        _emit(nc, tc, P_q, t)
    nc.compile()
    return nc


def _prep_core_inputs(hidden, sel_pad, Wq, bq, Wk, bk, Wv, bv, heads):
    """Build the in_map for one core. hidden: [S, HD] for this batch."""
    h0, h1, h2 = heads
    m = {}
    m["hT_full"] = np.ascontiguousarray(hidden.T.astype(np.float16))
    m["hT_sel"] = np.ascontiguousarray(hidden[sel_pad].T.astype(np.float16))

    def wT(Wmat, h):
        return np.ascontiguousarray(Wmat[h * D : (h + 1) * D, :].T)

    def bs(bvec, h):
        return bvec[h * D : (h + 1) * D]

    for nm, Wmat, bvec in (("q", Wq, bq), ("k", Wk, bk), ("v", Wv, bv)):
        m[f"w_{nm}01"] = np.concatenate([wT(Wmat, h0), wT(Wmat, h1)], axis=1)
        m[f"w_{nm}22"] = np.concatenate([wT(Wmat, h2), wT(Wmat, h2)], axis=1)
        m[f"b_{nm}01"] = np.concatenate([bs(bvec, h0), bs(bvec, h1)]).reshape(128, 1)
        m[f"b_{nm}22"] = np.concatenate([bs(bvec, h2), bs(bvec, h2)]).reshape(128, 1)
    m["ident"] = np.eye(128, dtype=np.float32)
    for k in list(m):
        dt = np.float16 if (k.startswith("w_") or k.startswith("hT_")) else np.float32
        m[k] = np.ascontiguousarray(m[k], dtype=dt)
    return m


def _plan(attention_mask):
    """Returns (P_q, sel list, sel_pad list)."""
    sels = [np.where(attention_mask[b] != 0)[0] for b in range(B)]
    nmax = max(1, max(len(s) for s in sels))
    P_q = ((nmax + 255) // 256) * 256
    sel_pads = []
    for s in sels:
        pad = np.zeros(P_q, dtype=np.int64)
        pad[: len(s)] = s
        sel_pads.append(pad)
    return P_q, sels, sel_pads


def build_in_maps(hidden_states, attention_mask, Wq, bq, Wk, bk, Wv, bv):
    P_q, sels, sel_pads = _plan(np.asarray(attention_mask))
    hs = np.asarray(hidden_states, dtype=np.float32)
    in_maps = []
    for c in range(NCORES):
        b, g = c // 4, c % 4
        heads = (3 * g, 3 * g + 1, 3 * g + 2)
        in_maps.append(
            _prep_core_inputs(hs[b], sel_pads[b], Wq, bq, Wk, bk, Wv, bv, heads)
        )
    return P_q, sels, in_maps


def assemble(results, P_q, sels, attention_mask):
    out = np.empty((B, S, HD), dtype=np.float32)
    mask = np.asarray(attention_mask)
    for c in range(NCORES):
        b, g = c // 4, c % 4
        r = results[c]["out"]
        cols = slice(192 * g, 192 * (g + 1))
        sel = sels[b]
        if len(sel):
            out[b, sel, cols] = r[: len(sel)]
        inv = np.where(mask[b] == 0)[0]
        if len(inv):
            out[b, inv, cols] = r[P_q]
    return out


def _install_ntff_shim():
    """Provide antenv.axon_hooks (missing from this image) so
    run_bass_kernel_spmd(trace=True) can capture NTFF profiles, and stub
    out the network-dependent artifact upload."""
    import types

    try:
        import antenv
    except ImportError:
        return
    try:
        from antenv.axon_hooks import get_axon_ntff_profile_hook  # noqa: F401
    except ImportError:
        try:
            if "/root/.axon_site" not in sys.path:
                sys.path.insert(0, "/root/.axon_site")
            from trn_agent_boot.trn_boot import _ntff_profile_via_ctypes

            hook = _ntff_profile_via_ctypes("/opt/axon/libaxon_pjrt.so")
        except Exception:
            hook = None
        mod = types.ModuleType("antenv.axon_hooks")
        _h = {"h": hook}
        mod.get_axon_ntff_profile_hook = lambda: _h["h"]
        mod.set_axon_ntff_profile_hook = lambda h: _h.__setitem__("h", h)
        sys.modules["antenv.axon_hooks"] = mod
        antenv.axon_hooks = mod

    _orig_upload = bass_utils.upload_artifacts

    def _safe_upload(tmpdir):
        try:
            return _orig_upload(tmpdir)
        except Exception:
            return tmpdir

    bass_utils.upload_artifacts = _safe_upload


def kernel(hidden_states, attention_mask, Wq, bq, Wk, bk, Wv, bv, trace=False):
    if trace:
        _install_ntff_shim()
    P_q, sels, in_maps = build_in_maps(
        hidden_states, attention_mask, Wq, bq, Wk, bk, Wv, bv
    )
    nc = _build(P_q)
    res = bass_utils.run_bass_kernel_spmd(
        nc, in_maps, core_ids=list(range(NCORES)), trace=trace
    )
    out = assemble(res.results, P_q, sels, attention_mask)
    if trace:
        kernel.last_exec_time_ns = res.exec_time_ns
        kernel.last_results = res
    return out


# revision 17
# speedup vs baseline: 2.8218x; 1.2065x over previous
"""BertSelfAttention (softsign-modified) Trainium2 Bass kernel.

Sharding: 8 cores = 2 batches x 4 head-groups (3 heads each).
Host gathers unmasked queries (mask applies along the QUERY dim only:
masked rows get uniform softmax => output = mean(V), filled host-side).

Device per core (all fp32):
  - proj: qT/kT/vT = W_hT.T @ hiddenT (hiddenT streamed in slabs)
  - k_mod = k/8 + softsign(softsign(k)/8) + v  (DVE fused ops;
    ss(ss1/8) = ss1/(8+|ss1|))
  - scores^T[k,q] = km^T.T @ qT  (two heads packed in partition halves,
    row-tiled concurrent matmuls)
  - probs = exp(scores/8) on ACT (no max subtraction needed; |s/8|<~8)
  - ctx^T[d,q] + sumexp via lhsT=[V|ones] accumulation over k tiles
  - PE-transpose ctx^T -> ctx, per-partition reciprocal normalize
  - meanV per head appended as the last output row
"""

import functools
import os
import sys

import numpy as np

for _p in ("/opt/trn_rl_repo", "/root/.axon_site/_ro/trn_rl_repo"):
    if os.path.isdir(_p) and _p not in sys.path:
        sys.path.append(_p)

import concourse.bacc as bacc
import concourse.mybir as mybir
import concourse.tile as tile
from concourse import bass_utils

F32 = mybir.dt.float32
BF16 = mybir.dt.float16  # 16-bit matmul dtype (fp16: 10-bit mantissa)
ALU = mybir.AluOpType
ACTF = mybir.ActivationFunctionType

B, S, HD, H, D = 2, 4096, 768, 12, 64
NCORES = 8
HPC = 3  # heads per core
QB = 512  # q block (one PSUM bank of fp32)
KT = 128  # k tile (partition dim of scores^T)
NB = 512  # projection N block
KCH = HD // 128  # 6 contraction chunks
NKT = S // KT  # 32 k tiles
SCALE = 0.125  # 1/sqrt(D)


def _qblocks(P_q):
    """Split P_q into blocks: 512s then one optional 256 tail."""
    out = []
    q0 = 0
    while P_q - q0 >= QB:
        out.append((q0, QB))
        q0 += QB
    if P_q - q0:
        out.append((q0, P_q - q0))
    return out


def _emit(nc, tc, P_q, t):
    """Emit the tile program. t = dict of dram tensor APs."""
    qbs = _qblocks(P_q)

    with (
        tc.tile_pool(name="persist", bufs=1) as P,
        tc.tile_pool(name="work", bufs=2) as W,
        tc.tile_pool(name="scr", bufs=5) as SCR,
        tc.tile_pool(name="probs", bufs=3) as PRB,
        tc.tile_pool(name="psA", bufs=2, space="PSUM") as psA,
        tc.tile_pool(name="psB", bufs=4, space="PSUM") as psB,
    ):
        # ---- persistent SBUF ----
        q01 = P.tile([128, P_q], BF16)
        q22 = P.tile([128, P_q], BF16)
        k01 = P.tile([128, S], F32)
        k22 = P.tile([128, S], F32)
        km01 = P.tile([128, S], BF16)
        km22 = P.tile([128, S], BF16)
        v01 = P.tile([128, S], F32)
        v22 = P.tile([128, S], F32)
        vn0 = P.tile([128, 65 * NKT], BF16)  # V natural + ones col, head 0
        vn1 = P.tile([128, 65 * NKT], BF16)
        vn2 = P.tile([128, 65 * NKT], BF16)
        outst = P.tile([128, (P_q // 128) * 192], F32)
        ident = P.tile([128, 128], F32)
        mvsb = P.tile([1, 192], F32)

        nc.sync.dma_start(ident[:], t["ident"][:])

        wsb = {}
        bsb = {}
        for nm in ("q01", "q22", "k01", "k22", "v01", "v22"):
            wsb[nm] = P.tile([128, KCH * 128], BF16, name=f"w_{nm}_sb")
            nc.sync.dma_start(
                wsb[nm][:].rearrange("p (c m) -> p c m", c=KCH),
                t[f"w_{nm}"][:].rearrange("(c p) m -> p c m", p=128),
            )
            bsb[nm] = P.tile([128, 1], F32, name=f"b_{nm}_sb")
            nc.sync.dma_start(bsb[nm][:], t[f"b_{nm}"][:])

        def proj_block(src_ap, blk, chains, on_act):
            n0, w = blk
            slab = W.tile([128, KCH * NB], BF16, tag="slab", name="slab")
            nc.sync.dma_start(
                slab[:, 0 : KCH * w].rearrange("p (c s) -> p c s", c=KCH),
                src_ap[:, n0 : n0 + w].rearrange("(c p) s -> p c s", p=128),
            )
            for nm, dst in chains:
                ps = psB.tile([128, NB], F32, tag="cx", name="pp")
                for c in range(KCH):
                    nc.tensor.matmul(
                        ps[:, 0:w],
                        wsb[nm][:, c * 128 : (c + 1) * 128],
                        slab[:, c * w : (c + 1) * w],
                        start=(c == 0),
                        stop=(c == KCH - 1),
                    )
                dsl = dst[:, n0 : n0 + w]
                if on_act:
                    nc.scalar.activation(
                        dsl, ps[:, 0:w], ACTF.Identity, bias=bsb[nm][:]
                    )
                else:
                    nc.vector.tensor_scalar_add(dsl, ps[:, 0:w], bsb[nm][:])

        def emit_vnat(vbuf, dsts, tts, on_act):
            for tt in tts:
                pt = psB.tile([128, 128], F32, tag="cx", name="pt")
                nc.tensor.transpose(pt[:], vbuf[:, tt * 128 : (tt + 1) * 128], ident[:])
                for vn, c0 in dsts:
                    dsl = vn[:, tt * 65 : tt * 65 + 64]
                    if on_act:
                        nc.scalar.copy(dsl, pt[:, c0 : c0 + 64])
                    else:
                        nc.vector.tensor_copy(dsl, pt[:, c0 : c0 + 64])

        CH = 1024
        U32 = mybir.dt.uint32

        def emit_km_chunk(kbuf, vbuf, kmbuf, ch, abs_on_act):
            sl = slice(ch * CH, (ch + 1) * CH)
            u = SCR.tile([128, CH], F32, tag="scr", name="u")
            nc.vector.scalar_tensor_tensor(
                u[:], kbuf[:, sl], SCALE, vbuf[:, sl], op0=ALU.mult, op1=ALU.add
            )
            a1 = SCR.tile([128, CH], F32, tag="scr", name="a1")
            if abs_on_act:
                nc.scalar.activation(a1[:], kbuf[:, sl], ACTF.Abs)
            else:
                nc.vector.tensor_scalar(
                    a1[:].bitcast(U32), kbuf[:, sl].bitcast(U32),
                    0x7FFFFFFF, None, op0=ALU.bitwise_and,
                )
            t1 = SCR.tile([128, CH], F32, tag="scr", name="t1")
            nc.vector.tensor_scalar_add(t1[:], a1[:], 1.0)
            r1 = SCR.tile([128, CH], F32, tag="scr", name="r1")
            nc.vector.reciprocal_approx_fast(r1[:], t1[:])
            s1 = SCR.tile([128, CH], F32, tag="scr", name="s1")
            nc.vector.tensor_mul(s1[:], kbuf[:, sl], r1[:])
            a2 = SCR.tile([128, CH], F32, tag="scr", name="a2")
            if abs_on_act:
                nc.scalar.activation(a2[:], s1[:], ACTF.Abs)
            else:
                nc.vector.tensor_scalar(
                    a2[:].bitcast(U32), s1[:].bitcast(U32),
                    0x7FFFFFFF, None, op0=ALU.bitwise_and,
                )
            t2 = SCR.tile([128, CH], F32, tag="scr", name="t2")
            nc.vector.tensor_scalar_add(t2[:], a2[:], 8.0)
            r2 = SCR.tile([128, CH], F32, tag="scr", name="r2")
            nc.vector.reciprocal_approx_fast(r2[:], t2[:])
            p1 = SCR.tile([128, CH], F32, tag="scr", name="p1")
            nc.vector.tensor_mul(p1[:], s1[:], r2[:])
            nc.vector.tensor_add(kmbuf[:, sl], u[:], p1[:])

        # ---- attention ----
        def epilogue(ctx, w, col0, q0):
            """ctx: PSUM [128, (w//128)*65] natural layout, col 64 = sumexp."""
            for j in range(w // 128):
                rc = W.tile([128, 1], F32, tag="rc", name="rc")
                nc.vector.reciprocal(rc[:], ctx[:, j * 65 + 64 : j * 65 + 65])
                g = q0 // 128 + j
                nc.vector.tensor_scalar_mul(
                    outst[:, g * 192 + col0 : g * 192 + col0 + 64],
                    ctx[:, j * 65 : j * 65 + 64],
                    rc[:],
                )

        def attn_block(kmbuf, qbuf, blkA, blkB, vnA, vnB, colA, colB):
            """One (slot0, slot1) pass over all k tiles; blk = (q0, width)."""
            qa, wa = blkA
            qb_, wb = blkB
            ctx0 = psB.tile([128, (QB // 128) * 65], F32, tag="cx", name="ctx0")
            ctx1 = psB.tile([128, (QB // 128) * 65], F32, tag="cx", name="ctx1")
            for kt in range(NKT):
                sc = psA.tile([128, 2 * QB], F32, tag="sc", name="sc")
                nc.tensor.matmul(
                    sc[:, 0:wa],
                    kmbuf[0:64, kt * KT : (kt + 1) * KT],
                    qbuf[0:64, qa : qa + wa],
                    start=True,
                    stop=True,
                )
                nc.tensor.matmul(
                    sc[:, QB : QB + wb],
                    kmbuf[64:128, kt * KT : (kt + 1) * KT],
                    qbuf[64:128, qb_ : qb_ + wb],
                    start=True,
                    stop=True,
                )
                pb = PRB.tile([128, 2 * QB], BF16, tag="pb", name="pb")
                if wa == QB:
                    nc.scalar.activation(pb[:, 0 : QB + wb], sc[:, 0 : QB + wb],
                                         ACTF.Exp, scale=SCALE)
                else:
                    nc.scalar.activation(pb[:, 0:wa], sc[:, 0:wa],
                                         ACTF.Exp, scale=SCALE)
                    nc.scalar.activation(pb[:, QB : QB + wb], sc[:, QB : QB + wb],
                                         ACTF.Exp, scale=SCALE)
                for j in range(wa // 128):
                    nc.tensor.matmul(
                        ctx0[:, j * 65 : (j + 1) * 65],
                        pb[:, j * 128 : (j + 1) * 128],
                        vnA[:, kt * 65 : kt * 65 + 65],
                        start=(kt == 0 and j == 0),
                        stop=(kt == NKT - 1 and j == wa // 128 - 1),
                    )
                for j in range(wb // 128):
                    nc.tensor.matmul(
                        ctx1[:, j * 65 : (j + 1) * 65],
                        pb[:, QB + j * 128 : QB + (j + 1) * 128],
                        vnB[:, kt * 65 : kt * 65 + 65],
                        start=(kt == 0 and j == 0),
                        stop=(kt == NKT - 1 and j == wb // 128 - 1),
                    )
            epilogue(ctx0, wa, colA, qa)
            epilogue(ctx1, wb, colB, qb_)

        def attn_tail(kmbuf, qbuf, blk, vn, col0):
            """Single q block, k tiles processed in row-tiled pairs."""
            qt, wt = blk
            ctx0 = psB.tile([128, (QB // 128) * 65], F32, tag="cx", name="ctxT")
            for k2 in range(NKT // 2):
                ka, kb = 2 * k2, 2 * k2 + 1
                sc = psA.tile([128, 2 * QB], F32, tag="sc", name="sc")
                nc.tensor.matmul(
                    sc[:, 0:wt],
                    kmbuf[0:64, ka * KT : (ka + 1) * KT],
                    qbuf[0:64, qt : qt + wt],
                    start=True,
                    stop=True,
                )
                nc.tensor.matmul(
                    sc[:, QB : QB + wt],
                    kmbuf[64:128, kb * KT : (kb + 1) * KT],
                    qbuf[64:128, qt : qt + wt],
                    start=True,
                    stop=True,
                )
                pb = PRB.tile([128, 2 * QB], BF16, tag="pb", name="pb")
                if wt == QB:
                    nc.scalar.activation(pb[:, 0 : 2 * QB], sc[:, 0 : 2 * QB],
                                         ACTF.Exp, scale=SCALE)
                else:
                    nc.scalar.activation(pb[:, 0:wt], sc[:, 0:wt],
                                         ACTF.Exp, scale=SCALE)
                    nc.scalar.activation(pb[:, QB : QB + wt], sc[:, QB : QB + wt],
                                         ACTF.Exp, scale=SCALE)
                for j in range(wt // 128):
                    nc.tensor.matmul(
                        ctx0[:, j * 65 : (j + 1) * 65],
                        pb[:, j * 128 : (j + 1) * 128],
                        vn[:, ka * 65 : ka * 65 + 65],
                        start=(k2 == 0 and j == 0),
                        stop=False,
                    )
                    nc.tensor.matmul(
                        ctx0[:, j * 65 : (j + 1) * 65],
                        pb[:, QB + j * 128 : QB + (j + 1) * 128],
                        vn[:, kb * 65 : kb * 65 + 65],
                        start=False,
                        stop=(k2 == NKT // 2 - 1 and j == wt // 128 - 1),
                    )
            epilogue(ctx0, wt, col0, qt)

        # ---- unit01 ingredients (ramp: bias/copies on ACT, abs on ACT) ----
        for vn in (vn0, vn1, vn2):
            nc.gpsimd.memset(vn[:], 1.0)

        for nb in range(S // NB):
            proj_block(
                t["hT_full"], (nb * NB, NB), [("k01", k01), ("v01", v01)], on_act=True
            )
        for ch in range(S // CH):
            emit_km_chunk(k01, v01, km01, ch, abs_on_act=True)
        emit_vnat(v01, [(vn0, 0), (vn1, 64)], range(NKT), on_act=True)
        for blk in qbs:
            proj_block(t["hT_sel"], blk, [("q01", q01)], on_act=True)

        # ---- unit22 ingredient slices, interleaved into unit01 attention
        # (ACT is saturated by exp there, so these use DVE/PE spare) ----
        u22_slices = []
        for i in range(4):
            sl = []
            sl.append(("kv", 2 * i))
            sl.append(("kv", 2 * i + 1))
            sl.append(("km", i))
            sl.append(("vn", i))
            u22_slices.append(sl)
        for i, blk in enumerate(qbs):
            u22_slices[i % 4].append(("q", blk))

        def emit_u22_slice(sl):
            for kind, i in sl:
                if kind == "kv":
                    proj_block(
                        t["hT_full"], (i * NB, NB), [("k22", k22), ("v22", v22)],
                        on_act=False,
                    )
                elif kind == "km":
                    emit_km_chunk(k22, v22, km22, i, abs_on_act=False)
                elif kind == "vn":
                    emit_vnat(v22, [(vn2, 0)], range(8 * i, 8 * i + 8), on_act=False)
                else:
                    proj_block(t["hT_sel"], i, [("q22", q22)], on_act=False)


        # unit01 attention with u22 ingredients interleaved
        for qi, blk in enumerate(qbs):
            attn_block(km01, q01, blk, blk, vn0, vn1, 0, 64)
            if qi < len(u22_slices):
                emit_u22_slice(u22_slices[qi])
        for sl in u22_slices[len(qbs) :]:
            emit_u22_slice(sl)

        # unit22: head 2 self-paired across q blocks
        for st in range(len(qbs) // 2):
            attn_block(
                km22, q22, qbs[2 * st], qbs[2 * st + 1], vn2, vn2, 128, 128
            )
        if len(qbs) % 2:
            attn_tail(km22, q22, qbs[-1], vn2, 128)

        # ---- meanV row ----
        for h, vn in ((0, vn0), (1, vn1), (2, vn2)):
            mv = psB.tile([1, 64], F32, tag="cx", name="mv")
            for kt in range(NKT):
                nc.tensor.matmul(
                    mv[:],
                    vn[:, kt * 65 + 64 : kt * 65 + 65],
                    vn[:, kt * 65 : kt * 65 + 64],
                    start=(kt == 0),
                    stop=(kt == NKT - 1),
                )
            nc.vector.tensor_scalar_mul(mvsb[:, h * 64 : (h + 1) * 64], mv[:], 1.0 / S)
        nc.sync.dma_start(t["out"][P_q : P_q + 1, :], mvsb[:])

        # ---- store ----
        for g in range(P_q // 128):
            nc.sync.dma_start(
                t["out"][g * 128 : (g + 1) * 128, :],
                outst[:, g * 192 : (g + 1) * 192],
            )


@functools.lru_cache(maxsize=4)
def _build(P_q):
    nc = bacc.Bacc(
        "TRN2",
        target_bir_lowering=False,
        debug=False,
        enable_asserts=False,
        num_devices=NCORES,
    )
    t = {}
    t["hT_full"] = nc.dram_tensor("hT_full", [HD, S], BF16, kind="ExternalInput").ap()
    t["hT_sel"] = nc.dram_tensor("hT_sel", [HD, P_q], BF16, kind="ExternalInput").ap()
    for nm in ("q01", "q22", "k01", "k22", "v01", "v22"):
        t[f"w_{nm}"] = nc.dram_tensor(
            f"w_{nm}", [HD, 128], BF16, kind="ExternalInput"
        ).ap()
        t[f"b_{nm}"] = nc.dram_tensor(
            f"b_{nm}", [128, 1], F32, kind="ExternalInput"
        ).ap()
    t["ident"] = nc.dram_tensor("ident", [128, 128], F32, kind="ExternalInput").ap()
    t["out"] = nc.dram_tensor("out", [P_q + 1, 192], F32, kind="ExternalOutput").ap()

    with tile.TileContext(nc) as tc:
        _emit(nc, tc, P_q, t)
    nc.compile()
    return nc


def _prep_core_inputs(hidden, sel_pad, Wq, bq, Wk, bk, Wv, bv, heads):
    """Build the in_map for one core. hidden: [S, HD] for this batch."""
    h0, h1, h2 = heads
    m = {}
    m["hT_full"] = np.ascontiguousarray(hidden.T.astype(np.float16))
    m["hT_sel"] = np.ascontiguousarray(hidden[sel_pad].T.astype(np.float16))

    def wT(Wmat, h):
        return np.ascontiguousarray(Wmat[h * D : (h + 1) * D, :].T)

    def bs(bvec, h):
        return bvec[h * D : (h + 1) * D]

    for nm, Wmat, bvec in (("q", Wq, bq), ("k", Wk, bk), ("v", Wv, bv)):
        m[f"w_{nm}01"] = np.concatenate([wT(Wmat, h0), wT(Wmat, h1)], axis=1)
        m[f"w_{nm}22"] = np.concatenate([wT(Wmat, h2), wT(Wmat, h2)], axis=1)
        m[f"b_{nm}01"] = np.concatenate([bs(bvec, h0), bs(bvec, h1)]).reshape(128, 1)
        m[f"b_{nm}22"] = np.concatenate([bs(bvec, h2), bs(bvec, h2)]).reshape(128, 1)
    m["ident"] = np.eye(128, dtype=np.float32)
    for k in list(m):
        dt = np.float16 if (k.startswith("w_") or k.startswith("hT_")) else np.float32
        m[k] = np.ascontiguousarray(m[k], dtype=dt)
    return m


def _plan(attention_mask):
    """Returns (P_q, sel list, sel_pad list)."""
    sels = [np.where(attention_mask[b] != 0)[0] for b in range(B)]
    nmax = max(1, max(len(s) for s in sels))
    P_q = ((nmax + 255) // 256) * 256
    sel_pads = []
    for s in sels:
        pad = np.zeros(P_q, dtype=np.int64)
        pad[: len(s)] = s
        sel_pads.append(pad)
    return P_q, sels, sel_pads


def build_in_maps(hidden_states, attention_mask, Wq, bq, Wk, bk, Wv, bv):
    P_q, sels, sel_pads = _plan(np.asarray(attention_mask))
    hs = np.asarray(hidden_states, dtype=np.float32)
    in_maps = []
    for c in range(NCORES):
        b, g = c // 4, c % 4
        heads = (3 * g, 3 * g + 1, 3 * g + 2)
        in_maps.append(
            _prep_core_inputs(hs[b], sel_pads[b], Wq, bq, Wk, bk, Wv, bv, heads)
        )
    return P_q, sels, in_maps


def assemble(results, P_q, sels, attention_mask):
    out = np.empty((B, S, HD), dtype=np.float32)
    mask = np.asarray(attention_mask)
    for c in range(NCORES):
        b, g = c // 4, c % 4
        r = results[c]["out"]
        cols = slice(192 * g, 192 * (g + 1))
        sel = sels[b]
        if len(sel):
            out[b, sel, cols] = r[: len(sel)]
        inv = np.where(mask[b] == 0)[0]
        if len(inv):
            out[b, inv, cols] = r[P_q]
    return out


def _install_ntff_shim():
    """Provide antenv.axon_hooks (missing from this image) so
    run_bass_kernel_spmd(trace=True) can capture NTFF profiles, and stub
    out the network-dependent artifact upload."""
    import types

    try:
        import antenv
    except ImportError:
        return
    try:
        from antenv.axon_hooks import get_axon_ntff_profile_hook  # noqa: F401
    except ImportError:
        try:
            if "/root/.axon_site" not in sys.path:
                sys.path.insert(0, "/root/.axon_site")
            from trn_agent_boot.trn_boot import _ntff_profile_via_ctypes

            hook = _ntff_profile_via_ctypes("/opt/axon/libaxon_pjrt.so")
        except Exception:
            hook = None
        mod = types.ModuleType("antenv.axon_hooks")
        _h = {"h": hook}
        mod.get_axon_ntff_profile_hook = lambda: _h["h"]
        mod.set_axon_ntff_profile_hook = lambda h: _h.__setitem__("h", h)
        sys.modules["antenv.axon_hooks"] = mod
        antenv.axon_hooks = mod

    _orig_upload = bass_utils.upload_artifacts

    def _safe_upload(tmpdir):
        try:
            return _orig_upload(tmpdir)
        except Exception:
            return tmpdir

    bass_utils.upload_artifacts = _safe_upload


def kernel(hidden_states, attention_mask, Wq, bq, Wk, bk, Wv, bv, trace=False):
    if trace:
        _install_ntff_shim()
    P_q, sels, in_maps = build_in_maps(
        hidden_states, attention_mask, Wq, bq, Wk, bk, Wv, bv
    )
    nc = _build(P_q)
    res = bass_utils.run_bass_kernel_spmd(
        nc, in_maps, core_ids=list(range(NCORES)), trace=trace
    )
    out = assemble(res.results, P_q, sels, attention_mask)
    if trace:
        kernel.last_exec_time_ns = res.exec_time_ns
        kernel.last_results = res
    return out
